# revision 1
# baseline (speedup 1.0000x reference)
"""Trainium2 kernel for nn_LocalEncoder (BLT-style local encoder).

Key structural insight: every per-token quantity (boundary logit z, rmsnorm
scale, q/k/v projections) depends only on the token ID (vocab=260), so all
dense math runs on the 260-row vocab tables instead of 16384 token rows.
Top-k boundary selection ties (same token id => bit-identical z in the fp32
reference) are broken by index, reproduced exactly on the host.

Pipeline:
  Kernel A (8 cores, DF split 8x384): zv partials = w2_slice @ silu(w1_slice @ embT)
  Host:     zv -> per-row boundary selection (stable by (-z, idx)) -> pos/pid/qtok
  Kernel B (8 cores = 4 seqs x 2 query-halves): one-hot gathers of vocab
            q/k/v tables per token, per-token scores + exp on DVE/ACT,
            block-diagonal softmax via one-hot scatter matmuls, wo proj.
"""

import os
import numpy as np
import ml_dtypes

import concourse.bass as bass
import concourse.bacc as bacc
import concourse.mybir as mybir
from concourse.tile import TileContext
from concourse.alu_op_type import AluOpType
from concourse.bass_utils import run_bass_kernel_spmd
from concourse import masks

F32 = mybir.dt.float32
F32R = mybir.dt.float32r
BF16 = mybir.dt.bfloat16
AFT = mybir.ActivationFunctionType
AX = mybir.AxisListType

B, L, D, V, K, H, HD = 4, 4096, 768, 260, 512, 12, 64
DF = 4 * D
VP = 384          # vocab padded to 3 partition chunks
RMS_EPS = 1e-5
NCORES = 8
FSL = DF // NCORES  # 384 f-rows per core in kernel A

_cache = {}


# --------------------------------------------------------------------------- #
# Kernel A: per-core partial zv over a DF slice (fp32 matmuls for precision)
# --------------------------------------------------------------------------- #
def build_kernel_a():
    nc = bacc.Bacc("TRN2", target_bir_lowering=False, debug=False)
    embT_d = nc.dram_tensor("embT", [D, V], F32, kind="ExternalInput")
    w1T_d = nc.dram_tensor("w1T", [D, FSL], F32, kind="ExternalInput")
    b1_d = nc.dram_tensor("b1s", [FSL], F32, kind="ExternalInput")
    w2_d = nc.dram_tensor("w2s", [FSL], F32, kind="ExternalInput")
    zp_d = nc.dram_tensor("zp", [1, V], F32, kind="ExternalOutput")

    with TileContext(nc) as tc:
        with (
            tc.tile_pool(name="sb", bufs=1) as sb,
            tc.tile_pool(name="ps", bufs=2, space="PSUM") as ps,
        ):
            embT = [sb.tile([128, V], F32, tag=f"embT{d}", name=f"embT{d}") for d in range(6)]
            w1T = [sb.tile([128, FSL], F32, tag=f"w1T{d}", name=f"w1T{d}") for d in range(6)]
            for d in range(6):
                nc.sync.dma_start(embT[d][:, :], embT_d[128 * d:128 * (d + 1), :])
                nc.sync.dma_start(w1T[d][:, :], w1T_d[128 * d:128 * (d + 1), :])
            b1c = sb.tile([128, 3], F32, tag="b1c")
            w2c = sb.tile([128, 3], F32, tag="w2c")
            nc.sync.dma_start(b1c[:, :], b1_d.rearrange("(i p) -> p i", p=128))
            nc.sync.dma_start(w2c[:, :], w2_d.rearrange("(i p) -> p i", p=128))

            zp_ps = ps.tile([1, V], F32, tag="zp")
            for fi in range(3):
                y1p = ps.tile([128, V], F32, tag="y1")
                for d in range(6):
                    nc.tensor.matmul(
                        y1p[:, :], w1T[d][:, 128 * fi:128 * (fi + 1)], embT[d][:, :],
                        start=(d == 0), stop=(d == 5),
                    )
                y1b = sb.tile([128, V], F32, tag="y1b")
                nc.vector.tensor_scalar(y1b[:, :], y1p[:, :], b1c[:, fi:fi + 1],
                                        None, AluOpType.add)
                sig = sb.tile([128, V], F32, tag="sig")
                nc.scalar.activation(sig[:, :], y1b[:, :], AFT.Sigmoid)
                y1s = sb.tile([128, V], F32, tag="y1s")
                nc.vector.tensor_tensor(y1s[:, :], y1b[:, :], sig[:, :],
                                        AluOpType.mult)
                nc.tensor.matmul(zp_ps[:, :], w2c[:, fi:fi + 1], y1s[:, :],
                                 start=(fi == 0), stop=(fi == 2))
            zp_s = sb.tile([1, V], F32, tag="zps")
            nc.vector.tensor_copy(zp_s[:, :], zp_ps[:, :])
            nc.sync.dma_start(zp_d[:, :], zp_s[:, :])
    nc.compile()
    return nc


def run_kernel_a(inputs):
    if "A" not in _cache:
        _cache["A"] = build_kernel_a()
    nc = _cache["A"]
    embT = np.ascontiguousarray(inputs["embed_W"].astype(np.float32).T)
    w1 = inputs["bp_w1"].astype(np.float32)
    b1 = inputs["bp_b1"].astype(np.float32)
    w2 = inputs["bp_w2"].astype(np.float32)[0]
    in_maps = []
    for c in range(NCORES):
        sl = slice(c * FSL, (c + 1) * FSL)
        in_maps.append({
            "embT": embT,
            "w1T": np.ascontiguousarray(w1[sl].T),
            "b1s": np.ascontiguousarray(b1[sl]),
            "w2s": np.ascontiguousarray(w2[sl]),
        })
    res = run_bass_kernel_spmd(nc, in_maps, list(range(NCORES)),
                               trace=os.environ.get("KERNEL_TRACE") == "1")
    _cache["tA"] = res.exec_time_ns
    zv = np.zeros(V, np.float64)
    for c in range(NCORES):
        zv += res.results[c]["zp"][0].astype(np.float64)
    zv += inputs["bp_b2"].astype(np.float64)[0]
    return zv.astype(np.float32)


# --------------------------------------------------------------------------- #
# Host boundary logic
# --------------------------------------------------------------------------- #
def boundary_plan(zv, tokens):
    """Reproduce reference top-k (stable ties by index) + patch structure."""
    zt = zv[tokens]  # [B, L]
    pos = np.zeros((B, K), np.int64)
    for b in range(B):
        key = zt[b].astype(np.float64).copy()
        key[0] = np.inf  # position 0 forced boundary (logprob set to 0 = max)
        order = np.lexsort((np.arange(L), -key))
        pos[b] = np.sort(order[:K])
    pid = (pos[:, None, :] <= np.arange(L)[None, :, None]).sum(-1) - 1  # [B, L]
    qtok = np.take_along_axis(tokens, np.take_along_axis(pos, pid, 1), 1)  # [B, L]
    return pos, pid, qtok


# --------------------------------------------------------------------------- #
# Kernel B: sparse cross-attention via vocab tables + one-hot matmuls
# --------------------------------------------------------------------------- #
def build_kernel_b(jobs):
    """jobs: per-core dict with j0 and per-jc tchunk ranges (python ints ->
    data-dependent instruction stream; same NEFF runs on all 8 cores with the
    max structure, masking handles core differences).  To keep one NEFF for
    all cores, we use the UNION structure: every core runs the same tchunk
    count per jc slot; tchunk indices and j0 are per-core DATA (iota bases
    must be static though) -- so instead we compile per-core variants only if
    structure differs.  Simpler: compile ONE program parameterized by the max
    chunk counts; per-core tchunk starts enter via DRAM-provided pid/tok/qtok
    columns (already per-core rebased by host).
    """
    n0, n1 = jobs["n0"], jobs["n1"]  # tchunks for jc0 / jc1 (uniform, padded)
    nc = bacc.Bacc("TRN2", target_bir_lowering=False, debug=False)

    # vocab tables
    emb_d = nc.dram_tensor("emb", [VP, D], F32, kind="ExternalInput")
    embT_d = nc.dram_tensor("embT", [D, VP], BF16, kind="ExternalInput")
    wqT_d = nc.dram_tensor("wqT", [D, D], BF16, kind="ExternalInput")
    wkT_d = nc.dram_tensor("wkT", [D, D], BF16, kind="ExternalInput")
    wvT_d = nc.dram_tensor("wvT", [D, D], BF16, kind="ExternalInput")
    woT_d = nc.dram_tensor("woT", [D, D], BF16, kind="ExternalInput")
    # per-core token structure, already sliced/padded by host:
    # rows: [1, NT*128] token ids / boundary-token ids (f32), NT = n0+n1
    NT = n0 + n1
    tqr_d = nc.dram_tensor("tqr", [1, NT * 256], F32R, kind="ExternalInput")
    pidc_d = nc.dram_tensor("pidc", [NT * 128], F32, kind="ExternalInput")
    out_d = nc.dram_tensor("out", [256, D], F32, kind="ExternalOutput")

    with TileContext(nc) as tc:
        with (
            tc.tile_pool(name="sb", bufs=1) as sb,
            tc.tile_pool(name="wk", bufs=1) as wkp,
            tc.tile_pool(name="ps", bufs=1, space="PSUM") as ps,
            tc.tile_pool(name="acc", bufs=1, space="PSUM") as accp,
        ):
            # ---- global small tiles ----
            ones_f = sb.tile([1, 128], F32, tag="onesf")
            nc.gpsimd.memset(ones_f[:, :], 1.0)
            ones_r = sb.tile([1, 128], F32R, tag="onesr")
            nc.vector.tensor_copy(ones_r[:, :], ones_f[:, :])
            ident = sb.tile([128, 128], F32, tag="ident")
            masks.make_identity(nc, ident[:, :])
            iotav = sb.tile([128, 3], F32, tag="iotav")  # col p+0/128/256
            for vc in range(3):
                nc.gpsimd.iota(iotav[:, vc:vc + 1], [[0, 1]], base=128 * vc,
                               channel_multiplier=1,
                               allow_small_or_imprecise_dtypes=True)
            iotaj = [sb.tile([128, 128], F32, tag=f"iotaj{jc}", name=f"iotaj{jc}") for jc in range(2)]
            for jc in range(2):
                # value = j0 + jc*128 + f ; j0 enters via host-rebased pid
                nc.gpsimd.iota(iotaj[jc][:, :], [[1, 128]], base=128 * jc,
                               channel_multiplier=0,
                               allow_small_or_imprecise_dtypes=True)

            # ---- load weights / tables ----
            emb = [sb.tile([128, D], F32, tag=f"emb{v}", name=f"emb{v}") for v in range(3)]
            for v in range(3):
                nc.sync.dma_start(emb[v][:, :], emb_d[128 * v:128 * (v + 1), :])
            embT = [sb.tile([128, VP], BF16, tag=f"embT{d}", name=f"embTb{d}") for d in range(6)]
            for d in range(6):
                nc.sync.dma_start(embT[d][:, :], embT_d[128 * d:128 * (d + 1), :])
            wts = {}
            for nm, dd in (("wq", wqT_d), ("wk", wkT_d), ("wv", wvT_d), ("wo", woT_d)):
                wts[nm] = [wkp.tile([128, D], BF16, tag=f"{nm}{d}", name=f"{nm}_{d}") for d in range(6)]
                for d in range(6):
                    nc.sync.dma_start(wts[nm][d][:, :], dd[128 * d:128 * (d + 1), :])

            # per-tchunk structure columns
            pidc = sb.tile([128, NT], F32, tag="pidc")
            nc.sync.dma_start(pidc[:, :], pidc_d.rearrange("(i p) -> p i", p=128))

            # ---- rmsnorm scales: rv (k/v), rv8 (q, includes /8) ----
            msq = sb.tile([128, 3], F32, tag="msq")
            sqjunk = sb.tile([128, D], F32, tag="sqjunk")
            for v in range(3):
                nc.scalar.activation(sqjunk[:, :], emb[v][:, :], AFT.Square,
                                     accum_out=msq[:, v:v + 1])
            # rv = (msq/768 + eps)^-1/2 = exp(-0.5*ln(msq/768 + eps))
            epsc = sb.tile([128, 1], F32, tag="epsc")
            nc.gpsimd.memset(epsc[:, :], RMS_EPS)
            lnv = sb.tile([128, 3], F32, tag="lnv")
            nc.scalar.activation(lnv[:, :], msq[:, :], AFT.Ln,
                                 scale=1.0 / D, bias=epsc[:, :1])
            rv = sb.tile([128, 3], F32, tag="rv")
            nc.scalar.activation(rv[:, :], lnv[:, :], AFT.Exp, scale=-0.5)
            rv8 = sb.tile([128, 3], F32, tag="rv8")
            nc.vector.tensor_scalar(rv8[:, :], rv[:, :], 0.125, None, AluOpType.mult)

            # ---- vocab tables q_s / k_n / v_n [3][128, D] f32r ----
            tabs = {}
            for nm, wname, scl in (("q", "wq", rv8), ("k", "wk", rv), ("v", "wv", rv)):
                tabs[nm] = []
                for v in range(3):
                    tp = ps.tile([128, D], F32, tag="qg", name="tp")
                    for d in range(6):
                        nc.tensor.matmul(
                            tp[:, :512], embT[d][:, 128 * v:128 * (v + 1)],
                            wts[wname][d][:, :512], start=(d == 0), stop=(d == 5))
                        nc.tensor.matmul(
                            tp[:, 512:], embT[d][:, 128 * v:128 * (v + 1)],
                            wts[wname][d][:, 512:], start=(d == 0), stop=(d == 5))
                    ts_ = sb.tile([128, D], BF16, tag=f"tab{nm}{v}")
                    nc.vector.tensor_scalar(ts_[:, :], tp[:, :], scl[:, v:v + 1],
                                            None, AluOpType.mult)
                    tabs[nm].append(ts_)

            # ---- main loop: two query chunks ----
            for jc in range(2):
                ntc = n0 if jc == 0 else n1
                base = 0 if jc == 0 else n0
                acc = accp.tile([128, 1536], F32, tag="acc", name="acc")
                for i in range(ntc):
                    tci = base + i
                    # broadcast token+qtok rows across partitions (one matmul)
                    tq_s = sb.tile([1, 256], F32R, tag="tokslice", name="tq_s", bufs=3)
                    nc.sync.dma_start(tq_s[:, :], tqr_d[:, 256 * tci:256 * (tci + 1)])
                    btok2 = ps.tile([128, 256], F32, tag="btok", name="btok2")
                    nc.tensor.matmul(btok2[:, :], ones_r[:, :], tq_s[:, :],
                                     start=True, stop=True)
                    btok = btok2[:, :128]
                    bqtok = btok2[:, 128:]
                    ohk = []
                    ohq = []
                    for v in range(3):
                        o1 = sb.tile([128, 128], BF16, tag=f"ohk{v}", name=f"o1_{v}", bufs=2)
                        nc.vector.tensor_scalar(o1[:, :], btok,
                                                iotav[:, v:v + 1], None,
                                                AluOpType.is_equal)
                        ohk.append(o1)
                        o2 = sb.tile([128, 128], BF16, tag=f"ohq{v}", name=f"o2_{v}", bufs=2)
                        nc.vector.tensor_scalar(o2[:, :], bqtok,
                                                iotav[:, v:v + 1], None,
                                                AluOpType.is_equal)
                        ohq.append(o2)
                    # gathers: qg/kg/vg [t,768]
                    qg = ps.tile([128, D], F32, tag="qg")
                    kg = ps.tile([128, D], F32, tag="kg")
                    for v in range(3):
                        nc.tensor.matmul(qg[:, :512], ohq[v][:, :],
                                         tabs["q"][v][:, :512],
                                         start=(v == 0), stop=(v == 2))
                        nc.tensor.matmul(qg[:, 512:], ohq[v][:, :],
                                         tabs["q"][v][:, 512:],
                                         start=(v == 0), stop=(v == 2))
                        nc.tensor.matmul(kg[:, :512], ohk[v][:, :],
                                         tabs["k"][v][:, :512],
                                         start=(v == 0), stop=(v == 2))
                        nc.tensor.matmul(kg[:, 512:], ohk[v][:, :],
                                         tabs["k"][v][:, 512:],
                                         start=(v == 0), stop=(v == 2))
                    # scores + exp
                    kgs = sb.tile([128, D], F32, tag="kgs", bufs=2)
                    nc.scalar.copy(kgs[:, :], kg[:, :])
                    prod = sb.tile([128, D], F32, tag="prod", bufs=2)
                    nc.vector.tensor_tensor(prod[:, :], qg[:, :], kgs[:, :],
                                            AluOpType.mult)
                    s12 = sb.tile([128, H], F32, tag="s12", bufs=2)
                    nc.vector.tensor_reduce(
                        ap3(s12, H, 1), ap3(prod, H, HD), AX.X, AluOpType.add)
                    e12f = sb.tile([128, H], F32, tag="e12f", bufs=2)
                    nc.scalar.activation(e12f[:, :], s12[:, :], AFT.Exp)
                    e12 = sb.tile([128, H], BF16, tag="e12", bufs=2)
                    nc.vector.tensor_copy(e12[:, :], e12f[:, :])
                    # value gather (reuses qg slot) and weight
                    vg = ps.tile([128, D], F32, tag="qg")
                    for v in range(3):
                        nc.tensor.matmul(vg[:, :512], ohk[v][:, :],
                                         tabs["v"][v][:, :512],
                                         start=(v == 0), stop=(v == 2))
                        nc.tensor.matmul(vg[:, 512:], ohk[v][:, :],
                                         tabs["v"][v][:, 512:],
                                         start=(v == 0), stop=(v == 2))
                    wv = sb.tile([128, D], BF16, tag="wv", bufs=2)
                    nc.vector.tensor_tensor(ap3(wv, H, HD),
                                            bcast3(e12f, H, HD),
                                            ap3(vg, H, HD), AluOpType.mult)
                    # membership MT [t, j] and scatter
                    mt = sb.tile([128, 128], BF16, tag="mt", bufs=2)
                    nc.vector.tensor_scalar(mt[:, :], iotaj[jc][:, :],
                                            pidc[:, tci:tci + 1], None,
                                            AluOpType.is_equal)
                    nc.tensor.matmul(acc[:, :512], mt[:, :], wv[:, :512],
                                     start=(i == 0), stop=(i == ntc - 1))
                    nc.tensor.matmul(acc[:, 512:768], mt[:, :], wv[:, 512:],
                                     start=(i == 0), stop=(i == ntc - 1))
                    nc.tensor.matmul(acc[:, 1024:1036], mt[:, :], e12[:, :],
                                     start=(i == 0), stop=(i == ntc - 1))
                # ---- finalize jc ----
                lnz = sb.tile([128, H], F32, tag="lnz")
                nc.scalar.activation(lnz[:, :], acc[:, 1024:1036], AFT.Ln)
                zrec = sb.tile([128, H], F32, tag="zrec")
                nc.scalar.activation(zrec[:, :], lnz[:, :], AFT.Exp, scale=-1.0)
                pr = sb.tile([128, D], F32, tag="pr")
                nc.vector.tensor_tensor(ap3(pr, H, HD), bcast3(zrec, H, HD),
                                        ap3(acc, H, HD, width=780), AluOpType.mult)
                fin = ps.tile([128, D], F32, tag="kg", name="fin")
                for d in range(6):
                    trp = ps.tile([128, 128], F32, tag="btok")
                    nc.tensor.transpose(trp[:, :], pr[:, 128 * d:128 * (d + 1)],
                                        ident[:, :])
                    trs = sb.tile([128, 128], BF16, tag="trs", bufs=2)
                    nc.vector.tensor_copy(trs[:, :], trp[:, :])
                    nc.tensor.matmul(fin[:, :512], trs[:, :], wts["wo"][d][:, :512],
                                     start=(d == 0), stop=(d == 5))
                    nc.tensor.matmul(fin[:, 512:], trs[:, :], wts["wo"][d][:, 512:],
                                     start=(d == 0), stop=(d == 5))
                fin_s = sb.tile([128, D], F32, tag="fins")
                nc.vector.tensor_copy(fin_s[:, :], fin[:, :])
                nc.sync.dma_start(out_d[128 * jc:128 * (jc + 1), :], fin_s[:, :])
    nc.compile()
    return nc


def ap3(tile, n, w, width=None):
    """[128, n*w] tile viewed as [128, n, w] (first n*w cols)."""
    p = tile.ap[0] if hasattr(tile, "ap") else None
    t = tile[:, :]
    ps, fs = t.ap[0], t.ap[1]
    return bass.AP(t.tensor, t.offset, [list(ps), [fs[0] * w, n], [fs[0], w]])


def bcast3(tile, n, w):
    """[128, n] tile broadcast to [128, n, w] via 0-stride inner dim."""
    t = tile[:, :]
    ps, fs = t.ap[0], t.ap[1]
    return bass.AP(t.tensor, t.offset, [list(ps), [fs[0], n], [0, w]])


# --------------------------------------------------------------------------- #
# top-level
# --------------------------------------------------------------------------- #
def kernel(tokens, embed_W, bp_w1, bp_b1, bp_w2, bp_b2, wq, wk, wv, wo,
           qnorm_w, kvnorm_w, k_patches):
    tokens = np.asarray(tokens).astype(np.int64)
    inputs = dict(tokens=tokens, embed_W=embed_W, bp_w1=bp_w1, bp_b1=bp_b1,
                  bp_w2=bp_w2, bp_b2=bp_b2)
    zv = run_kernel_a(inputs)
    pos, pid, qtok = boundary_plan(zv, tokens)

    # per-core job structure: core = 2*b + half; queries [half*256, half*256+256)
    cores = []
    for b in range(B):
        for half in range(2):
            j0 = half * 256
            ends = [pos[b, j0 + 128] if j0 + 128 < K else L,
                    pos[b, j0 + 256] if j0 + 256 < K else L]
            starts = [pos[b, j0], pos[b, j0 + 128] if j0 + 128 < K else L]
            tcs = []
            for jc in range(2):
                lo, hi = int(starts[jc]) // 128, -(-int(ends[jc]) // 128)
                tcs.append(list(range(lo, max(hi, lo + 1))))
            cores.append({"b": b, "j0": j0, "tcs": tcs})
    n0 = max(len(c["tcs"][0]) for c in cores)
    n1 = max(len(c["tcs"][1]) for c in cores)
    key = ("B", n0, n1)
    if key not in _cache:
        _cache[key] = build_kernel_b({"n0": n0, "n1": n1})
    nc = _cache[key]

    # host-side weight prep (norm-weight folding only)
    embp = np.zeros((VP, D), np.float32)
    embp[:V] = embed_W.astype(np.float32)
    embTp = np.ascontiguousarray(embp.T).astype(ml_dtypes.bfloat16)
    wq_f = np.ascontiguousarray((wq.astype(np.float32)
                                 * qnorm_w.astype(np.float32)[None, :]).T).astype(ml_dtypes.bfloat16)
    wk_f = np.ascontiguousarray((wk.astype(np.float32)
                                 * kvnorm_w.astype(np.float32)[None, :]).T).astype(ml_dtypes.bfloat16)
    wv_f = np.ascontiguousarray((wv.astype(np.float32)
                                 * kvnorm_w.astype(np.float32)[None, :]).T).astype(ml_dtypes.bfloat16)
    wo_f = np.ascontiguousarray(wo.astype(np.float32).T).astype(ml_dtypes.bfloat16)

    NT = n0 + n1
    in_maps = []
    for c in cores:
        b = c["b"]
        tqr = np.zeros(NT * 256, np.float32)
        pidc = np.full(NT * 128, -1.0, np.float32)  # -1 never matches a j id
        slot = 0
        for jc in range(2):
            lst = c["tcs"][jc]
            # pad each jc segment to its uniform length with repeats of the
            # first chunk (harmless: pid mask kills contributions, and for
            # padded slots we also set pid=-1)
            want = n0 if jc == 0 else n1
            for k_ in range(want):
                if k_ < len(lst):
                    tci = lst[k_]
                    sl = slice(tci * 128, (tci + 1) * 128)
                    tqr[slot * 256:slot * 256 + 128] = tokens[b, sl]
                    tqr[slot * 256 + 128:(slot + 1) * 256] = qtok[b, sl]
                    # rebase pid to local j index (0..255 within this core)
                    pidc[slot * 128:(slot + 1) * 128] = pid[b, sl] - c["j0"]
                slot += 1
        in_maps.append({
            "emb": embp, "embT": embTp, "wqT": wq_f, "wkT": wk_f,
            "wvT": wv_f, "woT": wo_f,
            "tqr": tqr[None, :], "pidc": pidc,
        })
    res = run_bass_kernel_spmd(nc, in_maps, list(range(NCORES)),
                               trace=os.environ.get("KERNEL_TRACE") == "1")
    _cache["tB"] = res.exec_time_ns
    out = np.zeros((B, K, D), np.float32)
    for ci, c in enumerate(cores):
        out[c["b"], c["j0"]:c["j0"] + 256] = res.results[ci]["out"]
    return out



# revision 7
# speedup vs baseline: 1.6692x; 1.6692x over previous
"""Trainium2 kernel for nn_LocalEncoder (BLT-style local encoder).

Key structural insight: every per-token quantity depends only on the token ID
(vocab=260), so the whole cross-attention collapses into vocab space:

  out_h(patch j) = sum_w C[w,j] * exp(S_h[w, qtok_j]) * vhat_h(w) / den
  den            = sum_w C[w,j] * exp(S_h[w, qtok_j])

with C[w,j] = count of tokens with id w inside patch j (host histogram),
S_h = khat_h^T qhat_h a (vocab x patch) score matrix, and qhat/khat/vhat the
vocab-space projection tables.  Device work per core is a handful of dense
vocab-sized matmuls -- no per-token gathers at all.

Pipeline:
  Kernel A (8 cores, DF split 8x384): zv partials = w2_slice @ silu(w1_slice @ embT)
  Host:     zv -> per-row boundary selection (stable by (-z, idx)) -> pos/pid,
            count matrix C[vocab, patch], qtok one-hot, folded weights
  Kernel B (8 cores = 4 seqs x 2 head-groups of 6): tables -> scores ->
            exp*count -> weighted-sum matmuls -> wo, partial outputs summed
            on host over the 2 head-groups.
"""

import os
import numpy as np
import ml_dtypes

import concourse.bass as bass
import concourse.bacc as bacc
import concourse.mybir as mybir
from concourse.tile import TileContext
from concourse.alu_op_type import AluOpType
from concourse.bass_utils import run_bass_kernel_spmd

F32 = mybir.dt.float32
BF16 = mybir.dt.bfloat16
AFT = mybir.ActivationFunctionType
AX = mybir.AxisListType

B, L, D, V, K, H, HD = 4, 4096, 768, 260, 512, 12, 64
DF = 4 * D
VP = 384          # vocab padded to 3 partition chunks
RMS_EPS = 1e-5
NCORES = 8
FSL = DF // NCORES  # 384 f-rows per core in kernel A
DG = 384            # head-group width (6 heads x 64)

_cache = {}


# --------------------------------------------------------------------------- #
# Kernel A: per-core partial zv over a DF slice (fp32 matmuls for precision)
# --------------------------------------------------------------------------- #
def build_kernel_a():
    nc = bacc.Bacc("TRN2", target_bir_lowering=False, debug=False)
    embT_d = nc.dram_tensor("embT", [D, V], F32, kind="ExternalInput")
    w1T_d = nc.dram_tensor("w1T", [D, FSL], F32, kind="ExternalInput")
    b1_d = nc.dram_tensor("b1s", [FSL], F32, kind="ExternalInput")
    w2_d = nc.dram_tensor("w2s", [FSL], F32, kind="ExternalInput")
    zp_d = nc.dram_tensor("zp", [1, V], F32, kind="ExternalOutput")

    with TileContext(nc) as tc:
        with (
            tc.tile_pool(name="sb", bufs=1) as sb,
            tc.tile_pool(name="ps", bufs=2, space="PSUM") as ps,
        ):
            embT = [sb.tile([128, V], F32, tag=f"embT{d}", name=f"embT{d}") for d in range(6)]
            w1T = [sb.tile([128, FSL], F32, tag=f"w1T{d}", name=f"w1T{d}") for d in range(6)]
            for d in range(6):
                nc.sync.dma_start(embT[d][:, :], embT_d[128 * d:128 * (d + 1), :])
                nc.sync.dma_start(w1T[d][:, :], w1T_d[128 * d:128 * (d + 1), :])
            b1c = sb.tile([128, 3], F32, tag="b1c")
            w2c = sb.tile([128, 3], F32, tag="w2c")
            nc.sync.dma_start(b1c[:, :], b1_d.rearrange("(i p) -> p i", p=128))
            nc.sync.dma_start(w2c[:, :], w2_d.rearrange("(i p) -> p i", p=128))

            zp_ps = ps.tile([1, V], F32, tag="zp")
            for fi in range(3):
                y1p = ps.tile([128, V], F32, tag="y1")
                for d in range(6):
                    nc.tensor.matmul(
                        y1p[:, :], w1T[d][:, 128 * fi:128 * (fi + 1)], embT[d][:, :],
                        start=(d == 0), stop=(d == 5),
                    )
                y1b = sb.tile([128, V], F32, tag="y1b")
                nc.vector.tensor_scalar(y1b[:, :], y1p[:, :], b1c[:, fi:fi + 1],
                                        None, AluOpType.add)
                sig = sb.tile([128, V], F32, tag="sig")
                nc.scalar.activation(sig[:, :], y1b[:, :], AFT.Sigmoid)
                y1s = sb.tile([128, V], F32, tag="y1s")
                nc.vector.tensor_tensor(y1s[:, :], y1b[:, :], sig[:, :],
                                        AluOpType.mult)
                nc.tensor.matmul(zp_ps[:, :], w2c[:, fi:fi + 1], y1s[:, :],
                                 start=(fi == 0), stop=(fi == 2))
            zp_s = sb.tile([1, V], F32, tag="zps")
            nc.vector.tensor_copy(zp_s[:, :], zp_ps[:, :])
            nc.sync.dma_start(zp_d[:, :], zp_s[:, :])
    nc.compile()
    return nc


def run_kernel_a(inputs):
    if "A" not in _cache:
        _cache["A"] = build_kernel_a()
    nc = _cache["A"]
    embT = np.ascontiguousarray(inputs["embed_W"].astype(np.float32).T)
    w1 = inputs["bp_w1"].astype(np.float32)
    b1 = inputs["bp_b1"].astype(np.float32)
    w2 = inputs["bp_w2"].astype(np.float32)[0]
    in_maps = []
    for c in range(NCORES):
        sl = slice(c * FSL, (c + 1) * FSL)
        in_maps.append({
            "embT": embT,
            "w1T": np.ascontiguousarray(w1[sl].T),
            "b1s": np.ascontiguousarray(b1[sl]),
            "w2s": np.ascontiguousarray(w2[sl]),
        })
    res = run_bass_kernel_spmd(nc, in_maps, list(range(NCORES)),
                               trace=os.environ.get("KERNEL_TRACE") == "1")
    _cache["tA"] = res.exec_time_ns
    zv = np.zeros(V, np.float64)
    for c in range(NCORES):
        zv += res.results[c]["zp"][0].astype(np.float64)
    zv += inputs["bp_b2"].astype(np.float64)[0]
    return zv.astype(np.float32)


# --------------------------------------------------------------------------- #
# Host boundary logic
# --------------------------------------------------------------------------- #
def boundary_plan(zv, tokens):
    """Reproduce reference top-k (stable ties by index) + patch structure."""
    zt = zv[tokens]  # [B, L]
    pos = np.zeros((B, K), np.int64)
    for b in range(B):
        key = zt[b].astype(np.float64).copy()
        key[0] = np.inf  # position 0 forced boundary (logprob set to 0 = max)
        order = np.lexsort((np.arange(L), -key))
        pos[b] = np.sort(order[:K])
    pid = (pos[:, None, :] <= np.arange(L)[None, :, None]).sum(-1) - 1  # [B, L]
    return pos, pid


# --------------------------------------------------------------------------- #
# Kernel B: count-matrix vocab-space cross attention, 6 heads per core
# --------------------------------------------------------------------------- #
def strided3(ap, n, w, stride, offset=0):
    """[128, *] AP viewed as [128, n, w] blocks at `offset` with block stride."""
    ps = ap.ap[0]
    return bass.AP(ap.tensor, ap.offset + offset,
                   [list(ps), [stride, n], [1, w]])


def build_kernel_b():
    nc = bacc.Bacc("TRN2", target_bir_lowering=False, debug=False)
    embT_d = nc.dram_tensor("embT", [D, VP], BF16, kind="ExternalInput")
    emb_d = nc.dram_tensor("emb", [VP, D], BF16, kind="ExternalInput")
    wqT_d = nc.dram_tensor("wqT", [D, DG], BF16, kind="ExternalInput")
    wkT_d = nc.dram_tensor("wkT", [D, DG], BF16, kind="ExternalInput")
    wvT_d = nc.dram_tensor("wvT", [D, DG], BF16, kind="ExternalInput")
    woT_d = nc.dram_tensor("woT", [DG, D], BF16, kind="ExternalInput")
    c_d = nc.dram_tensor("cnt", [VP, K], BF16, kind="ExternalInput")
    qoh_d = nc.dram_tensor("qoh", [VP, K], BF16, kind="ExternalInput")
    outT_d = nc.dram_tensor("outT", [D, K], F32, kind="ExternalOutput")

    with TileContext(nc) as tc:
        with (
            tc.tile_pool(name="sb", bufs=1) as sb,
            tc.tile_pool(name="ps", bufs=2, space="PSUM") as ps,
        ):
            # ---- loads ----
            embT6 = [sb.tile([128, VP], BF16, tag=f"embT{d}", name=f"embT{d}") for d in range(6)]
            wqT6 = [sb.tile([128, DG], BF16, tag=f"wqT{d}", name=f"wqT{d}") for d in range(6)]
            wkT6 = [sb.tile([128, DG], BF16, tag=f"wkT{d}", name=f"wkT{d}") for d in range(6)]
            wvT6 = [sb.tile([128, DG], BF16, tag=f"wvT{d}", name=f"wvT{d}") for d in range(6)]
            for d in range(6):
                sl = slice(128 * d, 128 * (d + 1))
                nc.sync.dma_start(embT6[d][:, :], embT_d[sl, :])
                nc.sync.dma_start(wqT6[d][:, :], wqT_d[sl, :])
                nc.sync.dma_start(wkT6[d][:, :], wkT_d[sl, :])
                nc.sync.dma_start(wvT6[d][:, :], wvT_d[sl, :])
            emb3 = [sb.tile([128, D], BF16, tag=f"emb{u}", name=f"emb{u}") for u in range(3)]
            C3 = [sb.tile([128, K], BF16, tag=f"C{u}", name=f"C{u}") for u in range(3)]
            QOH3 = [sb.tile([128, K], BF16, tag=f"QOH{u}", name=f"QOH{u}") for u in range(3)]
            woT3 = [sb.tile([128, D], BF16, tag=f"woT{u}", name=f"woT{u}") for u in range(3)]
            for u in range(3):
                sl = slice(128 * u, 128 * (u + 1))
                nc.sync.dma_start(emb3[u][:, :], emb_d[sl, :])
                nc.sync.dma_start(C3[u][:, :], c_d[sl, :])
                nc.sync.dma_start(QOH3[u][:, :], qoh_d[sl, :])
                nc.sync.dma_start(woT3[u][:, :], woT_d[sl, :])
            ones64 = sb.tile([128, 64], BF16, tag="ones64")
            nc.gpsimd.memset(ones64[:, :], 1.0)

            # ---- rmsnorm scales rv[u] = rsqrt(mean(emb_u^2) + eps) ----
            rv3 = []
            sqj = sb.tile([128, D], F32, tag="sqj", name="sqj")
            for u in range(3):
                msq = sb.tile([128, 1], F32, tag="msq", name="msq", bufs=2)
                nc.scalar.activation(sqj[:, :], emb3[u][:, :], AFT.Square,
                                     accum_out=msq[:, :])
                tn = sb.tile([128, 1], F32, tag="tn", name="tn", bufs=2)
                nc.vector.tensor_scalar(tn[:, :], msq[:, :], 1.0 / D, RMS_EPS,
                                        AluOpType.mult, AluOpType.add)
                tr = sb.tile([128, 1], F32, tag="tr", name="tr", bufs=2)
                nc.vector.reciprocal(tr[:, :], tn[:, :])
                rv = sb.tile([128, 1], F32, tag=f"rv{u}", name=f"rv{u}")
                nc.scalar.activation(rv[:, :], tr[:, :], AFT.Sqrt)
                rv3.append(rv)

            # ---- tables ----
            qhat3 = []   # [u-chunk][128, DG] rv-scaled (1/8 folded on host)
            for u in range(3):
                tp = ps.tile([128, K], F32, tag="w512", name="tpq")
                for d in range(6):
                    nc.tensor.matmul(tp[:, :DG], embT6[d][:, 128 * u:128 * (u + 1)],
                                     wqT6[d][:, :], start=(d == 0), stop=(d == 5))
                qs = sb.tile([128, DG], BF16, tag=f"qhat{u}", name=f"qhat{u}")
                nc.vector.tensor_scalar(qs[:, :], tp[:, :DG], rv3[u][:, 0:1],
                                        None, AluOpType.mult)
                qhat3.append(qs)
            ktT3 = []    # [r-chunk][128, VP] un-scaled (rv applied inside exp)
            for r in range(3):
                tp = ps.tile([128, K], F32, tag="w512", name="tpk")
                for d in range(6):
                    nc.tensor.matmul(tp[:, :VP], wkT6[d][:, 128 * r:128 * (r + 1)],
                                     embT6[d][:, :], start=(d == 0), stop=(d == 5))
                ks = sb.tile([128, VP], BF16, tag=f"ktT{r}", name=f"ktT{r}")
                nc.scalar.copy(ks[:, :], tp[:, :VP])
                ktT3.append(ks)
            vhat3 = []   # [u-chunk][128, 390]: per head 64 cols + ones col
            for u in range(3):
                tp = ps.tile([128, K], F32, tag="w512", name="tpv")
                for d in range(6):
                    nc.tensor.matmul(tp[:, :DG], embT6[d][:, 128 * u:128 * (u + 1)],
                                     wvT6[d][:, :], start=(d == 0), stop=(d == 5))
                vt = sb.tile([128, 390], BF16, tag=f"vhat{u}", name=f"vhat{u}")
                nc.gpsimd.memset(strided3(vt[:, :], 6, 1, 65, offset=64), 1.0)
                nc.vector.tensor_scalar(strided3(vt[:, :], 6, 64, 65),
                                        strided3(tp[:, :DG], 6, 64, 64),
                                        rv3[u][:, 0:1], None, AluOpType.mult)
                vhat3.append(vt)

            # ---- qgT[r, j]: q rows gathered per patch by qtok one-hot ----
            qgT3 = []
            for r in range(3):
                qp = ps.tile([128, K], F32, tag="w512", name="qp")
                for u in range(3):
                    nc.tensor.matmul(qp[:, :], qhat3[u][:, 128 * r:128 * (r + 1)],
                                     QOH3[u][:, :], start=(u == 0), stop=(u == 2))
                qg = sb.tile([128, K], BF16, tag=f"qgT{r}", name=f"qgT{r}")
                nc.vector.tensor_copy(qg[:, :], qp[:, :])
                qgT3.append(qg)

            # ---- attention: per head scores -> exp*count -> num/den ----
            prT3 = [sb.tile([128, K], BF16, tag=f"prT{r}", name=f"prT{r}") for r in range(3)]
            for h in range(6):
                r, off = h // 2, 64 * (h % 2)
                nm = ps.tile([128, K], F32, tag="num", name="nm")
                for w in range(3):
                    sp = ps.tile([128, K], F32, tag="w512", name="sp")
                    nc.tensor.matmul(sp[:, :],
                                     ktT3[r][off:off + 64, 128 * w:128 * (w + 1)],
                                     qgT3[r][off:off + 64, :],
                                     start=True, stop=True)
                    ex = sb.tile([128, K], BF16, tag="ex", name="ex", bufs=3)
                    nc.scalar.activation(ex[:, :], sp[:, :], AFT.Exp,
                                         scale=rv3[w][:, 0:1])
                    xt = sb.tile([128, K], BF16, tag="xt", name="xt", bufs=3)
                    nc.vector.tensor_tensor(xt[:, :], ex[:, :], C3[w][:, :],
                                            AluOpType.mult)
                    nc.tensor.matmul(nm[0:65, :], vhat3[w][:, 65 * h:65 * h + 65],
                                     xt[:, :], start=(w == 0), stop=(w == 2))
                rd = sb.tile([128, K], BF16, tag="rd", name="rd", bufs=2)
                with nc.allow_low_precision("bf16 softmax denominator"):
                    nc.vector.reciprocal(rd[64:65, :], nm[64:65, :])
                pp = ps.tile([128, K], F32, tag="pp", name="pp")
                nc.tensor.matmul(pp[0:64, :], ones64[64:65, :], rd[64:65, :],
                                 start=True, stop=True)
                nmsb = sb.tile([64, K], BF16, tag="nmsb", name="nmsb", bufs=2)
                nc.scalar.copy(nmsb[:, :], nm[0:64, :])
                if off == 0:
                    nc.vector.tensor_tensor(prT3[r][0:64, :], nmsb[:, :],
                                            pp[0:64, :], AluOpType.mult)
                else:
                    po = sb.tile([64, K], BF16, tag="po", name="po", bufs=2)
                    nc.vector.tensor_tensor(po[:, :], nmsb[:, :], pp[0:64, :],
                                            AluOpType.mult)
                    nc.sync.dma_start(prT3[r][64:128, :], po[:, :])

            # ---- wo projection (transposed output) ----
            for m in range(6):
                op = ps.tile([128, K], F32, tag="w512", name="op")
                for kc in range(3):
                    nc.tensor.matmul(op[:, :], woT3[kc][:, 128 * m:128 * (m + 1)],
                                     prT3[kc][:, :], start=(kc == 0), stop=(kc == 2))
                ot = sb.tile([128, K], F32, tag="ot", name="ot", bufs=2)
                nc.vector.tensor_copy(ot[:, :], op[:, :])
                nc.sync.dma_start(outT_d[128 * m:128 * (m + 1), :], ot[:, :])
    nc.compile()
    return nc


# --------------------------------------------------------------------------- #
# top-level
# --------------------------------------------------------------------------- #
def kernel(tokens, embed_W, bp_w1, bp_b1, bp_w2, bp_b2, wq, wk, wv, wo,
           qnorm_w, kvnorm_w, k_patches):
    tokens = np.asarray(tokens).astype(np.int64)
    inputs = dict(tokens=tokens, embed_W=embed_W, bp_w1=bp_w1, bp_b1=bp_b1,
                  bp_w2=bp_w2, bp_b2=bp_b2)
    zv = run_kernel_a(inputs)
    pos, pid = boundary_plan(zv, tokens)
    qtokp = np.take_along_axis(tokens, pos, 1)  # [B, K] boundary token ids

    if "B" not in _cache:
        _cache["B"] = build_kernel_b()
    nc = _cache["B"]

    bf16 = ml_dtypes.bfloat16
    embp = np.zeros((VP, D), np.float32)
    embp[:V] = embed_W.astype(np.float32)
    emb_s = embp.astype(bf16)
    embT_s = np.ascontiguousarray(embp.T).astype(bf16)
    wqT_full = np.ascontiguousarray(
        (wq.astype(np.float32) * qnorm_w.astype(np.float32)[None, :]).T / 8.0)
    wkT_full = np.ascontiguousarray(
        (wk.astype(np.float32) * kvnorm_w.astype(np.float32)[None, :]).T)
    wvT_full = np.ascontiguousarray(
        (wv.astype(np.float32) * kvnorm_w.astype(np.float32)[None, :]).T)
    woT_full = np.ascontiguousarray(wo.astype(np.float32).T)

    in_maps = []
    for b in range(B):
        C = np.zeros((VP, K), np.float32)
        np.add.at(C, (tokens[b], pid[b]), 1.0)
        QOH = np.zeros((VP, K), np.float32)
        QOH[qtokp[b], np.arange(K)] = 1.0
        C_s = C.astype(bf16)
        QOH_s = QOH.astype(bf16)
        for g in range(2):
            cols = slice(DG * g, DG * (g + 1))
            in_maps.append({
                "embT": embT_s, "emb": emb_s,
                "wqT": np.ascontiguousarray(wqT_full[:, cols]).astype(bf16),
                "wkT": np.ascontiguousarray(wkT_full[:, cols]).astype(bf16),
                "wvT": np.ascontiguousarray(wvT_full[:, cols]).astype(bf16),
                "woT": np.ascontiguousarray(woT_full[cols, :]).astype(bf16),
                "cnt": C_s, "qoh": QOH_s,
            })
    res = run_bass_kernel_spmd(nc, in_maps, list(range(NCORES)),
                               trace=os.environ.get("KERNEL_TRACE") == "1")
    _cache["tB"] = res.exec_time_ns
    out = np.zeros((B, K, D), np.float32)
    for b in range(B):
        outT = res.results[2 * b]["outT"] + res.results[2 * b + 1]["outT"]
        out[b] = outT.T
    return out


# revision 11
# speedup vs baseline: 1.6965x; 1.0163x over previous
"""Trainium2 kernel for nn_LocalEncoder (BLT-style local encoder).

Key structural insight: every per-token quantity depends only on the token ID
(vocab=260), so the whole cross-attention collapses into vocab space:

  out_h(patch j) = sum_w C[w,j] * exp(S_h[w, qtok_j]) * vhat_h(w) / den
  den            = sum_w C[w,j] * exp(S_h[w, qtok_j])

with C[w,j] = count of tokens with id w inside patch j (host histogram),
S_h = khat_h^T qhat_h a (vocab x patch) score matrix, and qhat/khat/vhat the
vocab-space projection tables.  Device work per core is a handful of dense
vocab-sized matmuls -- no per-token gathers at all.

Pipeline:
  Kernel A (8 cores, DF split 8x384): zv partials = w2_slice @ silu(w1_slice @ embT)
  Host:     zv -> per-row boundary selection (stable by (-z, idx)) -> pos/pid,
            count matrix C[vocab, patch], qtok one-hot, folded weights
  Kernel B (8 cores = 4 seqs x 2 head-groups of 6): tables -> scores ->
            exp*count -> weighted-sum matmuls -> wo, partial outputs summed
            on host over the 2 head-groups.
"""

import os
import numpy as np
import ml_dtypes

import concourse.bass as bass
import concourse.bacc as bacc
import concourse.mybir as mybir
from concourse.tile import TileContext
from concourse.alu_op_type import AluOpType
from concourse.bass_utils import run_bass_kernel_spmd

F32 = mybir.dt.float32
F32R = mybir.dt.float32r
BF16 = mybir.dt.bfloat16
AFT = mybir.ActivationFunctionType
AX = mybir.AxisListType

B, L, D, V, K, H, HD = 4, 4096, 768, 260, 512, 12, 64
DF = 4 * D
VP = 384          # vocab padded to 3 partition chunks
RMS_EPS = 1e-5
NCORES = 8
FSL = DF // NCORES  # 384 f-rows per core in kernel A
DG = 384            # head-group width (6 heads x 64)

_cache = {}


# --------------------------------------------------------------------------- #
# Kernel A: per-core partial zv over a DF slice (fp32 matmuls for precision)
# --------------------------------------------------------------------------- #
def build_kernel_a():
    nc = bacc.Bacc("TRN2", target_bir_lowering=False, debug=False)
    embT_d = nc.dram_tensor("embT", [D, V], F32R, kind="ExternalInput")
    w1T_d = nc.dram_tensor("w1T", [D, FSL], F32R, kind="ExternalInput")
    b1_d = nc.dram_tensor("b1s", [FSL], F32, kind="ExternalInput")
    w2_d = nc.dram_tensor("w2s", [FSL], F32, kind="ExternalInput")
    zp_d = nc.dram_tensor("zp", [1, V], F32, kind="ExternalOutput")

    with TileContext(nc) as tc:
        with (
            tc.tile_pool(name="sb", bufs=1) as sb,
            tc.tile_pool(name="ps", bufs=2, space="PSUM") as ps,
        ):
            embT = [sb.tile([128, V], F32R, tag=f"embT{d}", name=f"embT{d}") for d in range(6)]
            w1T = [sb.tile([128, FSL], F32R, tag=f"w1T{d}", name=f"w1T{d}") for d in range(6)]
            for d in range(6):
                nc.sync.dma_start(embT[d][:, :], embT_d[128 * d:128 * (d + 1), :])
                nc.sync.dma_start(w1T[d][:, :], w1T_d[128 * d:128 * (d + 1), :])
            b1c = sb.tile([128, 3], F32, tag="b1c")
            w2c = sb.tile([128, 3], F32, tag="w2c")
            nc.sync.dma_start(b1c[:, :], b1_d.rearrange("(i p) -> p i", p=128))
            nc.sync.dma_start(w2c[:, :], w2_d.rearrange("(i p) -> p i", p=128))

            zp_ps = ps.tile([1, V], F32, tag="zp")
            for fi in range(3):
                y1p = ps.tile([128, V], F32, tag="y1")
                for d in range(6):
                    nc.tensor.matmul(
                        y1p[:, :], w1T[d][:, 128 * fi:128 * (fi + 1)], embT[d][:, :],
                        start=(d == 0), stop=(d == 5),
                    )
                y1b = sb.tile([128, V], F32, tag="y1b")
                nc.vector.tensor_scalar(y1b[:, :], y1p[:, :], b1c[:, fi:fi + 1],
                                        None, AluOpType.add)
                sig = sb.tile([128, V], F32, tag="sig")
                nc.scalar.activation(sig[:, :], y1b[:, :], AFT.Sigmoid)
                y1s = sb.tile([128, V], F32, tag="y1s")
                nc.vector.tensor_tensor(y1s[:, :], y1b[:, :], sig[:, :],
                                        AluOpType.mult)
                nc.tensor.matmul(zp_ps[:, :], w2c[:, fi:fi + 1], y1s[:, :],
                                 start=(fi == 0), stop=(fi == 2))
            zp_s = sb.tile([1, V], F32, tag="zps")
            nc.vector.tensor_copy(zp_s[:, :], zp_ps[:, :])
            nc.sync.dma_start(zp_d[:, :], zp_s[:, :])
    nc.compile()
    return nc


def run_kernel_a(inputs):
    if "A" not in _cache:
        _cache["A"] = build_kernel_a()
    nc = _cache["A"]
    embT = np.ascontiguousarray(inputs["embed_W"].astype(np.float32).T)
    w1 = inputs["bp_w1"].astype(np.float32)
    b1 = inputs["bp_b1"].astype(np.float32)
    w2 = inputs["bp_w2"].astype(np.float32)[0]
    in_maps = []
    for c in range(NCORES):
        sl = slice(c * FSL, (c + 1) * FSL)
        in_maps.append({
            "embT": embT,
            "w1T": np.ascontiguousarray(w1[sl].T),
            "b1s": np.ascontiguousarray(b1[sl]),
            "w2s": np.ascontiguousarray(w2[sl]),
        })
    res = run_bass_kernel_spmd(nc, in_maps, list(range(NCORES)),
                               trace=os.environ.get("KERNEL_TRACE") == "1")
    _cache["tA"] = res.exec_time_ns
    zv = np.zeros(V, np.float64)
    for c in range(NCORES):
        zv += res.results[c]["zp"][0].astype(np.float64)
    zv += inputs["bp_b2"].astype(np.float64)[0]
    return zv.astype(np.float32)


# --------------------------------------------------------------------------- #
# Host boundary logic
# --------------------------------------------------------------------------- #
def boundary_plan(zv, tokens):
    """Reproduce reference top-k (stable ties by index) + patch structure."""
    zt = zv[tokens]  # [B, L]
    pos = np.zeros((B, K), np.int64)
    for b in range(B):
        key = zt[b].astype(np.float64).copy()
        key[0] = np.inf  # position 0 forced boundary (logprob set to 0 = max)
        order = np.lexsort((np.arange(L), -key))
        pos[b] = np.sort(order[:K])
    pid = (pos[:, None, :] <= np.arange(L)[None, :, None]).sum(-1) - 1  # [B, L]
    return pos, pid


# --------------------------------------------------------------------------- #
# Kernel B: count-matrix vocab-space cross attention, 6 heads per core
# --------------------------------------------------------------------------- #
def strided3(ap, n, w, stride, offset=0):
    """[128, *] AP viewed as [128, n, w] blocks at `offset` with block stride."""
    ps = ap.ap[0]
    return bass.AP(ap.tensor, ap.offset + offset,
                   [list(ps), [stride, n], [1, w]])


def build_kernel_b():
    nc = bacc.Bacc("TRN2", target_bir_lowering=False, debug=False)
    embT_d = nc.dram_tensor("embT", [D, VP], BF16, kind="ExternalInput")
    emb_d = nc.dram_tensor("emb", [VP, D], BF16, kind="ExternalInput")
    wqT_d = nc.dram_tensor("wqT", [D, DG], BF16, kind="ExternalInput")
    wkT_d = nc.dram_tensor("wkT", [D, DG], BF16, kind="ExternalInput")
    wvT_d = nc.dram_tensor("wvT", [D, DG], BF16, kind="ExternalInput")
    woT_d = nc.dram_tensor("woT", [DG, D], BF16, kind="ExternalInput")
    c_d = nc.dram_tensor("cnt", [VP, K], BF16, kind="ExternalInput")
    qoh_d = nc.dram_tensor("qoh", [VP, K], BF16, kind="ExternalInput")
    outT_d = nc.dram_tensor("outT", [D, K], F32, kind="ExternalOutput")

    with TileContext(nc) as tc:
        with (
            tc.tile_pool(name="sb", bufs=1) as sb,
            tc.tile_pool(name="ps", bufs=2, space="PSUM") as ps,
        ):
            # ---- loads ----
            embT6 = [sb.tile([128, VP], BF16, tag=f"embT{d}", name=f"embT{d}") for d in range(6)]
            wqT6 = [sb.tile([128, DG], BF16, tag=f"wqT{d}", name=f"wqT{d}") for d in range(6)]
            wkT6 = [sb.tile([128, DG], BF16, tag=f"wkT{d}", name=f"wkT{d}") for d in range(6)]
            wvT6 = [sb.tile([128, DG], BF16, tag=f"wvT{d}", name=f"wvT{d}") for d in range(6)]
            for d in range(6):
                sl = slice(128 * d, 128 * (d + 1))
                nc.sync.dma_start(embT6[d][:, :], embT_d[sl, :])
                nc.sync.dma_start(wqT6[d][:, :], wqT_d[sl, :])
                nc.sync.dma_start(wkT6[d][:, :], wkT_d[sl, :])
                nc.sync.dma_start(wvT6[d][:, :], wvT_d[sl, :])
            emb3 = [sb.tile([128, D], BF16, tag=f"emb{u}", name=f"emb{u}") for u in range(3)]
            C3 = [sb.tile([128, K], BF16, tag=f"C{u}", name=f"C{u}") for u in range(3)]
            QOH3 = [sb.tile([128, K], BF16, tag=f"QOH{u}", name=f"QOH{u}") for u in range(3)]
            woT3 = [sb.tile([128, D], BF16, tag=f"woT{u}", name=f"woT{u}") for u in range(3)]
            for u in range(3):
                sl = slice(128 * u, 128 * (u + 1))
                nc.sync.dma_start(emb3[u][:, :], emb_d[sl, :])
                nc.sync.dma_start(C3[u][:, :], c_d[sl, :])
                nc.sync.dma_start(QOH3[u][:, :], qoh_d[sl, :])
                nc.sync.dma_start(woT3[u][:, :], woT_d[sl, :])
            ones64 = sb.tile([128, 64], BF16, tag="ones64")
            nc.gpsimd.memset(ones64[:, :], 1.0)

            # ---- rmsnorm scales rv[u] = rsqrt(mean(emb_u^2) + eps) ----
            rv3 = []
            for u in range(3):
                sq = sb.tile([128, D], BF16, tag="sq", name="sq", bufs=2)
                nc.gpsimd.tensor_tensor(sq[:, :], emb3[u][:, :], emb3[u][:, :],
                                        AluOpType.mult)
                msq = sb.tile([128, 1], F32, tag="msq", name="msq", bufs=2)
                nc.vector.tensor_reduce(msq[:, :], sq[:, :], AX.X, AluOpType.add)
                tn = sb.tile([128, 1], F32, tag="tn", name="tn", bufs=2)
                nc.vector.tensor_scalar(tn[:, :], msq[:, :], 1.0 / D, RMS_EPS,
                                        AluOpType.mult, AluOpType.add)
                tr = sb.tile([128, 1], F32, tag="tr", name="tr", bufs=2)
                nc.vector.reciprocal(tr[:, :], tn[:, :])
                rv = sb.tile([128, 1], F32, tag=f"rv{u}", name=f"rv{u}")
                nc.scalar.activation(rv[:, :], tr[:, :], AFT.Sqrt)
                rv3.append(rv)

            # ---- tables ----
            qhat3 = []   # [u-chunk][128, DG] rv-scaled (1/8 folded on host)
            for u in range(3):
                tp = ps.tile([128, K], F32, tag="w512", name="tpq")
                for d in range(6):
                    nc.tensor.matmul(tp[:, :DG], embT6[d][:, 128 * u:128 * (u + 1)],
                                     wqT6[d][:, :], start=(d == 0), stop=(d == 5))
                qs = sb.tile([128, DG], BF16, tag=f"qhat{u}", name=f"qhat{u}")
                nc.vector.tensor_scalar(qs[:, :], tp[:, :DG], rv3[u][:, 0:1],
                                        None, AluOpType.mult)
                qhat3.append(qs)
            ktT3 = []    # [r-chunk][128, VP] un-scaled (rv applied inside exp)
            for r in range(3):
                tp = ps.tile([128, K], F32, tag="w512", name="tpk")
                for d in range(6):
                    nc.tensor.matmul(tp[:, :VP], wkT6[d][:, 128 * r:128 * (r + 1)],
                                     embT6[d][:, :], start=(d == 0), stop=(d == 5))
                ks = sb.tile([128, VP], BF16, tag=f"ktT{r}", name=f"ktT{r}")
                nc.scalar.copy(ks[:, :], tp[:, :VP])
                ktT3.append(ks)
            vhat3 = []   # [u-chunk][128, 390]: per head 64 cols + ones col
            for u in range(3):
                tp = ps.tile([128, K], F32, tag="w512", name="tpv")
                for d in range(6):
                    nc.tensor.matmul(tp[:, :DG], embT6[d][:, 128 * u:128 * (u + 1)],
                                     wvT6[d][:, :], start=(d == 0), stop=(d == 5))
                vt = sb.tile([128, 390], BF16, tag=f"vhat{u}", name=f"vhat{u}")
                nc.gpsimd.memset(strided3(vt[:, :], 6, 1, 65, offset=64), 1.0)
                nc.vector.tensor_scalar(strided3(vt[:, :], 6, 64, 65),
                                        strided3(tp[:, :DG], 6, 64, 64),
                                        rv3[u][:, 0:1], None, AluOpType.mult)
                vhat3.append(vt)

            # ---- qgT[r, j]: q rows gathered per patch by qtok one-hot ----
            qgT3 = []
            for r in range(3):
                qp = ps.tile([128, K], F32, tag="w512", name="qp")
                for u in range(3):
                    nc.tensor.matmul(qp[:, :], qhat3[u][:, 128 * r:128 * (r + 1)],
                                     QOH3[u][:, :], start=(u == 0), stop=(u == 2))
                qg = sb.tile([128, K], BF16, tag=f"qgT{r}", name=f"qgT{r}")
                nc.vector.tensor_copy(qg[:, :], qp[:, :])
                qgT3.append(qg)

            # ---- attention: per head scores -> exp*count -> num/den ----
            prT3 = [sb.tile([128, K], BF16, tag=f"prT{r}", name=f"prT{r}") for r in range(3)]
            for h in range(6):
                r, off = h // 2, 64 * (h % 2)
                nm = ps.tile([128, K], F32, tag="num", name="nm")
                for w in range(3):
                    sp = ps.tile([128, K], F32, tag="w512", name="sp")
                    nc.tensor.matmul(sp[:, :],
                                     ktT3[r][off:off + 64, 128 * w:128 * (w + 1)],
                                     qgT3[r][off:off + 64, :],
                                     start=True, stop=True)
                    ex = sb.tile([128, K], BF16, tag="ex", name="ex", bufs=3)
                    nc.scalar.activation(ex[:, :], sp[:, :], AFT.Exp,
                                         scale=rv3[w][:, 0:1])
                    xt = sb.tile([128, K], BF16, tag="xt", name="xt", bufs=3)
                    nc.vector.tensor_tensor(xt[:, :], ex[:, :], C3[w][:, :],
                                            AluOpType.mult)
                    nc.tensor.matmul(nm[0:65, :], vhat3[w][:, 65 * h:65 * h + 65],
                                     xt[:, :], start=(w == 0), stop=(w == 2))
                rd = sb.tile([128, K], BF16, tag="rd", name="rd", bufs=2)
                with nc.allow_low_precision("bf16 softmax denominator"):
                    nc.vector.reciprocal(rd[64:65, :], nm[64:65, :])
                pp = ps.tile([128, K], F32, tag="pp", name="pp")
                nc.tensor.matmul(pp[0:64, :], ones64[64:65, :], rd[64:65, :],
                                 start=True, stop=True)
                nmsb = sb.tile([64, K], BF16, tag="nmsb", name="nmsb", bufs=2)
                nc.scalar.copy(nmsb[:, :], nm[0:64, :])
                if off == 0:
                    nc.vector.tensor_tensor(prT3[r][0:64, :], nmsb[:, :],
                                            pp[0:64, :], AluOpType.mult)
                else:
                    po = sb.tile([64, K], BF16, tag="po", name="po", bufs=2)
                    nc.vector.tensor_tensor(po[:, :], nmsb[:, :], pp[0:64, :],
                                            AluOpType.mult)
                    nc.sync.dma_start(prT3[r][64:128, :], po[:, :])

            # ---- wo projection (transposed output) ----
            for m in range(6):
                op = ps.tile([128, K], F32, tag="w512", name="op")
                for kc in range(3):
                    nc.tensor.matmul(op[:, :], woT3[kc][:, 128 * m:128 * (m + 1)],
                                     prT3[kc][:, :], start=(kc == 0), stop=(kc == 2))
                ot = sb.tile([128, K], F32, tag="ot", name="ot", bufs=2)
                nc.vector.tensor_copy(ot[:, :], op[:, :])
                nc.sync.dma_start(outT_d[128 * m:128 * (m + 1), :], ot[:, :])
    nc.compile()
    return nc


# --------------------------------------------------------------------------- #
# top-level
# --------------------------------------------------------------------------- #
def kernel(tokens, embed_W, bp_w1, bp_b1, bp_w2, bp_b2, wq, wk, wv, wo,
           qnorm_w, kvnorm_w, k_patches):
    tokens = np.asarray(tokens).astype(np.int64)
    inputs = dict(tokens=tokens, embed_W=embed_W, bp_w1=bp_w1, bp_b1=bp_b1,
                  bp_w2=bp_w2, bp_b2=bp_b2)
    zv = run_kernel_a(inputs)
    pos, pid = boundary_plan(zv, tokens)
    qtokp = np.take_along_axis(tokens, pos, 1)  # [B, K] boundary token ids

    if "B" not in _cache:
        _cache["B"] = build_kernel_b()
    nc = _cache["B"]

    bf16 = ml_dtypes.bfloat16
    embp = np.zeros((VP, D), np.float32)
    embp[:V] = embed_W.astype(np.float32)
    emb_s = embp.astype(bf16)
    embT_s = np.ascontiguousarray(embp.T).astype(bf16)
    wqT_full = np.ascontiguousarray(
        (wq.astype(np.float32) * qnorm_w.astype(np.float32)[None, :]).T / 8.0)
    wkT_full = np.ascontiguousarray(
        (wk.astype(np.float32) * kvnorm_w.astype(np.float32)[None, :]).T)
    wvT_full = np.ascontiguousarray(
        (wv.astype(np.float32) * kvnorm_w.astype(np.float32)[None, :]).T)
    woT_full = np.ascontiguousarray(wo.astype(np.float32).T)

    in_maps = []
    for b in range(B):
        C = np.zeros((VP, K), np.float32)
        np.add.at(C, (tokens[b], pid[b]), 1.0)
        QOH = np.zeros((VP, K), np.float32)
        QOH[qtokp[b], np.arange(K)] = 1.0
        C_s = C.astype(bf16)
        QOH_s = QOH.astype(bf16)
        for g in range(2):
            cols = slice(DG * g, DG * (g + 1))
            in_maps.append({
                "embT": embT_s, "emb": emb_s,
                "wqT": np.ascontiguousarray(wqT_full[:, cols]).astype(bf16),
                "wkT": np.ascontiguousarray(wkT_full[:, cols]).astype(bf16),
                "wvT": np.ascontiguousarray(wvT_full[:, cols]).astype(bf16),
                "woT": np.ascontiguousarray(woT_full[cols, :]).astype(bf16),
                "cnt": C_s, "qoh": QOH_s,
            })
    res = run_bass_kernel_spmd(nc, in_maps, list(range(NCORES)),
                               trace=os.environ.get("KERNEL_TRACE") == "1")
    _cache["tB"] = res.exec_time_ns
    out = np.zeros((B, K, D), np.float32)
    for b in range(B):
        outT = res.results[2 * b]["outT"] + res.results[2 * b + 1]["outT"]
        out[b] = outT.T
    return out


# revision 13
# speedup vs baseline: 1.7998x; 1.0609x over previous
"""Trainium2 kernel for nn_LocalEncoder (BLT-style local encoder).

Key structural insight: every per-token quantity depends only on the token ID
(vocab=260), so the whole cross-attention collapses into vocab space:

  out_h(patch j) = sum_w C[w,j] * exp(S_h[w, qtok_j]) * vhat_h(w) / den
  den            = sum_w C[w,j] * exp(S_h[w, qtok_j])

with C[w,j] = count of tokens with id w inside patch j (host histogram),
S_h = khat_h^T qhat_h a (vocab x patch) score matrix, and qhat/khat/vhat the
vocab-space projection tables.  Device work per core is a handful of dense
vocab-sized matmuls -- no per-token gathers at all.

Pipeline:
  Kernel A (8 cores, DF split 8x384): zv partials = w2_slice @ silu(w1_slice @ embT)
  Host:     zv -> per-row boundary selection (stable by (-z, idx)) -> pos/pid,
            count matrix C[vocab, patch], qtok one-hot, folded weights
  Kernel B (8 cores = 4 seqs x 2 head-groups of 6): tables -> scores ->
            exp*count -> weighted-sum matmuls -> wo, partial outputs summed
            on host over the 2 head-groups.
"""

import os
import numpy as np
import ml_dtypes

import concourse.bass as bass
import concourse.bacc as bacc
import concourse.mybir as mybir
from concourse.tile import TileContext
from concourse.alu_op_type import AluOpType
from concourse.bass_utils import run_bass_kernel_spmd

F32 = mybir.dt.float32
F32R = mybir.dt.float32r
BF16 = mybir.dt.bfloat16
AFT = mybir.ActivationFunctionType
AX = mybir.AxisListType

B, L, D, V, K, H, HD = 4, 4096, 768, 260, 512, 12, 64
DF = 4 * D
VP = 384          # vocab padded to 3 partition chunks
RMS_EPS = 1e-5
NCORES = 8
FSL = DF // NCORES  # 384 f-rows per core in kernel A
DG = 384            # head-group width (6 heads x 64)

_cache = {}


# --------------------------------------------------------------------------- #
# Kernel A: per-core partial zv over a DF slice (fp32 matmuls for precision)
# --------------------------------------------------------------------------- #
def build_kernel_a():
    nc = bacc.Bacc("TRN2", target_bir_lowering=False, debug=False)
    embT_d = nc.dram_tensor("embT", [128, 6 * V], F32R, kind="ExternalInput")
    w1T_d = nc.dram_tensor("w1T", [128, 6 * FSL], F32R, kind="ExternalInput")
    bw_d = nc.dram_tensor("bw", [128, 6], F32, kind="ExternalInput")
    zp_d = nc.dram_tensor("zp", [1, V], F32, kind="ExternalOutput")

    with TileContext(nc) as tc:
        with (
            tc.tile_pool(name="sb", bufs=1) as sb,
            tc.tile_pool(name="ps", bufs=2, space="PSUM") as ps,
        ):
            embT_t = sb.tile([128, 6 * V], F32R, tag="embT", name="embT_t")
            w1T_t = sb.tile([128, 6 * FSL], F32R, tag="w1T", name="w1T_t")
            bw = sb.tile([128, 6], F32, tag="bw", name="bw")
            nc.sync.dma_start(embT_t[:, :], embT_d[:, :])
            nc.sync.dma_start(w1T_t[:, :], w1T_d[:, :])
            nc.sync.dma_start(bw[:, :], bw_d[:, :])
            embT = [embT_t[:, V * d:V * (d + 1)] for d in range(6)]
            w1T = [w1T_t[:, FSL * d:FSL * (d + 1)] for d in range(6)]
            b1c = bw[:, 0:3]
            w2c = bw[:, 3:6]

            zp_ps = ps.tile([1, V], F32, tag="zp")
            for fi in range(3):
                y1p = ps.tile([128, V], F32, tag="y1")
                for d in range(6):
                    nc.tensor.matmul(
                        y1p[:, :], w1T_t[:, FSL * d + 128 * fi:FSL * d + 128 * (fi + 1)],
                        embT[d], start=(d == 0), stop=(d == 5),
                    )
                y1b = sb.tile([128, V], F32, tag="y1b")
                nc.vector.tensor_scalar(y1b[:, :], y1p[:, :], b1c[:, fi:fi + 1],
                                        None, AluOpType.add)
                sig = sb.tile([128, V], F32, tag="sig")
                nc.scalar.activation(sig[:, :], y1b[:, :], AFT.Sigmoid)
                y1s = sb.tile([128, V], F32, tag="y1s")
                nc.vector.tensor_tensor(y1s[:, :], y1b[:, :], sig[:, :],
                                        AluOpType.mult)
                nc.tensor.matmul(zp_ps[:, :], w2c[:, fi:fi + 1], y1s[:, :],
                                 start=(fi == 0), stop=(fi == 2))
            zp_s = sb.tile([1, V], F32, tag="zps")
            nc.vector.tensor_copy(zp_s[:, :], zp_ps[:, :])
            nc.sync.dma_start(zp_d[:, :], zp_s[:, :])
    nc.compile()
    return nc


def run_kernel_a(inputs):
    if "A" not in _cache:
        _cache["A"] = build_kernel_a()
    nc = _cache["A"]
    embT = inputs["embed_W"].astype(np.float32).T          # [768, 260]
    embT_r = np.ascontiguousarray(
        embT.reshape(6, 128, V).transpose(1, 0, 2).reshape(128, 6 * V))
    w1 = inputs["bp_w1"].astype(np.float32)
    b1 = inputs["bp_b1"].astype(np.float32)
    w2 = inputs["bp_w2"].astype(np.float32)[0]
    in_maps = []
    for c in range(NCORES):
        sl = slice(c * FSL, (c + 1) * FSL)
        w1T_r = np.ascontiguousarray(
            w1[sl].T.reshape(6, 128, FSL).transpose(1, 0, 2).reshape(128, 6 * FSL))
        bw = np.zeros((128, 6), np.float32)
        bw[:, 0:3] = b1[sl].reshape(3, 128).T
        bw[:, 3:6] = w2[sl].reshape(3, 128).T
        in_maps.append({"embT": embT_r, "w1T": w1T_r, "bw": bw})
    res = run_bass_kernel_spmd(nc, in_maps, list(range(NCORES)),
                               trace=os.environ.get("KERNEL_TRACE") == "1")
    _cache["tA"] = res.exec_time_ns
    zv = np.zeros(V, np.float64)
    for c in range(NCORES):
        zv += res.results[c]["zp"][0].astype(np.float64)
    zv += inputs["bp_b2"].astype(np.float64)[0]
    return zv.astype(np.float32)


# --------------------------------------------------------------------------- #
# Host boundary logic
# --------------------------------------------------------------------------- #
def boundary_plan(zv, tokens):
    """Reproduce reference top-k (stable ties by index) + patch structure."""
    zt = zv[tokens]  # [B, L]
    pos = np.zeros((B, K), np.int64)
    for b in range(B):
        key = zt[b].astype(np.float64).copy()
        key[0] = np.inf  # position 0 forced boundary (logprob set to 0 = max)
        order = np.lexsort((np.arange(L), -key))
        pos[b] = np.sort(order[:K])
    pid = (pos[:, None, :] <= np.arange(L)[None, :, None]).sum(-1) - 1  # [B, L]
    return pos, pid


# --------------------------------------------------------------------------- #
# Kernel B: count-matrix vocab-space cross attention, 6 heads per core
# --------------------------------------------------------------------------- #
def strided3(ap, n, w, stride, offset=0):
    """[128, *] AP viewed as [128, n, w] blocks at `offset` with block stride."""
    ps = ap.ap[0]
    return bass.AP(ap.tensor, ap.offset + offset,
                   [list(ps), [stride, n], [1, w]])


def build_kernel_b():
    nc = bacc.Bacc("TRN2", target_bir_lowering=False, debug=False)
    embT_d = nc.dram_tensor("embT", [128, 6 * VP], BF16, kind="ExternalInput")
    emb_d = nc.dram_tensor("emb", [128, 3 * D], BF16, kind="ExternalInput")
    wqT_d = nc.dram_tensor("wqT", [128, 6 * DG], BF16, kind="ExternalInput")
    wkT_d = nc.dram_tensor("wkT", [128, 6 * DG], BF16, kind="ExternalInput")
    wvT_d = nc.dram_tensor("wvT", [128, 6 * DG], BF16, kind="ExternalInput")
    woT_d = nc.dram_tensor("woT", [128, 3 * D], BF16, kind="ExternalInput")
    c_d = nc.dram_tensor("cnt", [128, 3 * K], BF16, kind="ExternalInput")
    qoh_d = nc.dram_tensor("qoh", [128, 3 * K], BF16, kind="ExternalInput")
    outT_d = nc.dram_tensor("outT", [D, K], F32, kind="ExternalOutput")

    with TileContext(nc) as tc:
        with (
            tc.tile_pool(name="sb", bufs=1) as sb,
            tc.tile_pool(name="ps", bufs=2, space="PSUM") as ps,
        ):
            # ---- loads (one DMA per logical tensor) ----
            embT_t = sb.tile([128, 6 * VP], BF16, tag="embT", name="embT_t")
            wqT_t = sb.tile([128, 6 * DG], BF16, tag="wqT", name="wqT_t")
            wkT_t = sb.tile([128, 6 * DG], BF16, tag="wkT", name="wkT_t")
            wvT_t = sb.tile([128, 6 * DG], BF16, tag="wvT", name="wvT_t")
            emb_t = sb.tile([128, 3 * D], BF16, tag="emb", name="emb_t")
            c_t = sb.tile([128, 3 * K], BF16, tag="ct", name="c_t")
            qoh_t = sb.tile([128, 3 * K], BF16, tag="qoht", name="qoh_t")
            wo_t = sb.tile([128, 3 * D], BF16, tag="wot", name="wo_t")
            nc.sync.dma_start(embT_t[:, :], embT_d[:, :])
            nc.sync.dma_start(wqT_t[:, :], wqT_d[:, :])
            nc.sync.dma_start(wkT_t[:, :], wkT_d[:, :])
            nc.sync.dma_start(qoh_t[:, :], qoh_d[:, :])
            nc.sync.dma_start(wvT_t[:, :], wvT_d[:, :])
            nc.sync.dma_start(emb_t[:, :], emb_d[:, :])
            nc.sync.dma_start(c_t[:, :], c_d[:, :])
            nc.sync.dma_start(wo_t[:, :], woT_d[:, :])
            embT6 = [embT_t[:, VP * d:VP * (d + 1)] for d in range(6)]
            wqT6 = [wqT_t[:, DG * d:DG * (d + 1)] for d in range(6)]
            wkT6 = [wkT_t[:, DG * d:DG * (d + 1)] for d in range(6)]
            wvT6 = [wvT_t[:, DG * d:DG * (d + 1)] for d in range(6)]
            emb3 = [emb_t[:, D * u:D * (u + 1)] for u in range(3)]
            C3 = [c_t[:, K * u:K * (u + 1)] for u in range(3)]
            QOH3 = [qoh_t[:, K * u:K * (u + 1)] for u in range(3)]
            woT3 = [wo_t[:, D * u:D * (u + 1)] for u in range(3)]
            ones64 = sb.tile([128, 64], BF16, tag="ones64")
            nc.gpsimd.memset(ones64[:, :], 1.0)

            # ---- rmsnorm scales rv[u] = rsqrt(mean(emb_u^2) + eps) ----
            rv3 = []
            for u in range(3):
                sq = sb.tile([128, D], BF16, tag="sq", name="sq", bufs=2)
                nc.gpsimd.tensor_tensor(sq[:, :], emb3[u][:, :], emb3[u][:, :],
                                        AluOpType.mult)
                msq = sb.tile([128, 1], F32, tag="msq", name="msq", bufs=2)
                nc.vector.tensor_reduce(msq[:, :], sq[:, :], AX.X, AluOpType.add)
                tn = sb.tile([128, 1], F32, tag="tn", name="tn", bufs=2)
                nc.vector.tensor_scalar(tn[:, :], msq[:, :], 1.0 / D, RMS_EPS,
                                        AluOpType.mult, AluOpType.add)
                tr = sb.tile([128, 1], F32, tag="tr", name="tr", bufs=2)
                nc.vector.reciprocal(tr[:, :], tn[:, :])
                rv = sb.tile([128, 1], F32, tag=f"rv{u}", name=f"rv{u}")
                nc.scalar.activation(rv[:, :], tr[:, :], AFT.Sqrt)
                rv3.append(rv)

            # ---- tables ----
            qhat3 = []   # [u-chunk][128, DG] rv-scaled (1/8 folded on host)
            for u in range(3):
                tp = ps.tile([128, K], F32, tag="w512", name="tpq")
                for d in range(6):
                    nc.tensor.matmul(tp[:, :DG], embT6[d][:, 128 * u:128 * (u + 1)],
                                     wqT6[d][:, :], start=(d == 0), stop=(d == 5))
                qs = sb.tile([128, DG], BF16, tag=f"qhat{u}", name=f"qhat{u}")
                nc.vector.tensor_scalar(qs[:, :], tp[:, :DG], rv3[u][:, 0:1],
                                        None, AluOpType.mult)
                qhat3.append(qs)
            ktT3 = []    # [r-chunk][128, VP] un-scaled (rv applied inside exp)
            for r in range(3):
                tp = ps.tile([128, K], F32, tag="w512", name="tpk")
                for d in range(6):
                    nc.tensor.matmul(tp[:, :VP], wkT6[d][:, 128 * r:128 * (r + 1)],
                                     embT6[d][:, :], start=(d == 0), stop=(d == 5))
                ks = sb.tile([128, VP], BF16, tag=f"ktT{r}", name=f"ktT{r}")
                nc.scalar.copy(ks[:, :], tp[:, :VP])
                ktT3.append(ks)
            vhat3 = []   # [u-chunk][128, 390]: per head 64 cols + ones col
            for u in range(3):
                tp = ps.tile([128, K], F32, tag="w512", name="tpv")
                for d in range(6):
                    nc.tensor.matmul(tp[:, :DG], embT6[d][:, 128 * u:128 * (u + 1)],
                                     wvT6[d][:, :], start=(d == 0), stop=(d == 5))
                vt = sb.tile([128, 390], BF16, tag=f"vhat{u}", name=f"vhat{u}")
                nc.gpsimd.memset(strided3(vt[:, :], 6, 1, 65, offset=64), 1.0)
                nc.vector.tensor_scalar(strided3(vt[:, :], 6, 64, 65),
                                        strided3(tp[:, :DG], 6, 64, 64),
                                        rv3[u][:, 0:1], None, AluOpType.mult)
                vhat3.append(vt)

            # ---- qgT[r, j]: q rows gathered per patch by qtok one-hot ----
            qgT3 = []
            for r in range(3):
                qp = ps.tile([128, K], F32, tag="w512", name="qp")
                for u in range(3):
                    nc.tensor.matmul(qp[:, :], qhat3[u][:, 128 * r:128 * (r + 1)],
                                     QOH3[u][:, :], start=(u == 0), stop=(u == 2))
                qg = sb.tile([128, K], BF16, tag=f"qgT{r}", name=f"qgT{r}")
                nc.vector.tensor_copy(qg[:, :], qp[:, :])
                qgT3.append(qg)

            # ---- attention: per head scores -> exp*count -> num/den ----
            prT3 = [sb.tile([128, K], BF16, tag=f"prT{r}", name=f"prT{r}") for r in range(3)]
            for h in range(6):
                r, off = h // 2, 64 * (h % 2)
                nm = ps.tile([128, K], F32, tag="num", name="nm")
                for w in range(3):
                    sp = ps.tile([128, K], F32, tag="w512", name="sp")
                    nc.tensor.matmul(sp[:, :],
                                     ktT3[r][off:off + 64, 128 * w:128 * (w + 1)],
                                     qgT3[r][off:off + 64, :],
                                     start=True, stop=True)
                    ex = sb.tile([128, K], BF16, tag="ex", name="ex", bufs=3)
                    nc.scalar.activation(ex[:, :], sp[:, :], AFT.Exp,
                                         scale=rv3[w][:, 0:1])
                    xt = sb.tile([128, K], BF16, tag="xt", name="xt", bufs=3)
                    nc.vector.tensor_tensor(xt[:, :], ex[:, :], C3[w][:, :],
                                            AluOpType.mult)
                    nc.tensor.matmul(nm[0:65, :], vhat3[w][:, 65 * h:65 * h + 65],
                                     xt[:, :], start=(w == 0), stop=(w == 2))
                rd = sb.tile([128, K], BF16, tag="rd", name="rd", bufs=2)
                with nc.allow_low_precision("bf16 softmax denominator"):
                    nc.vector.reciprocal(rd[64:65, :], nm[64:65, :])
                pp = ps.tile([128, K], F32, tag="pp", name="pp")
                nc.tensor.matmul(pp[0:64, :], ones64[64:65, :], rd[64:65, :],
                                 start=True, stop=True)
                nmsb = sb.tile([64, K], BF16, tag="nmsb", name="nmsb", bufs=2)
                nc.scalar.copy(nmsb[:, :], nm[0:64, :])
                if off == 0:
                    nc.vector.tensor_tensor(prT3[r][0:64, :], nmsb[:, :],
                                            pp[0:64, :], AluOpType.mult)
                else:
                    po = sb.tile([64, K], BF16, tag="po", name="po", bufs=2)
                    nc.vector.tensor_tensor(po[:, :], nmsb[:, :], pp[0:64, :],
                                            AluOpType.mult)
                    nc.sync.dma_start(prT3[r][64:128, :], po[:, :])

            # ---- wo projection (transposed output) ----
            for m in range(6):
                op = ps.tile([128, K], F32, tag="w512", name="op")
                for kc in range(3):
                    nc.tensor.matmul(op[:, :], woT3[kc][:, 128 * m:128 * (m + 1)],
                                     prT3[kc][:, :], start=(kc == 0), stop=(kc == 2))
                ot = sb.tile([128, K], F32, tag="ot", name="ot", bufs=2)
                nc.vector.tensor_copy(ot[:, :], op[:, :])
                nc.sync.dma_start(outT_d[128 * m:128 * (m + 1), :], ot[:, :])
    nc.compile()
    return nc


# --------------------------------------------------------------------------- #
# top-level
# --------------------------------------------------------------------------- #
def kernel(tokens, embed_W, bp_w1, bp_b1, bp_w2, bp_b2, wq, wk, wv, wo,
           qnorm_w, kvnorm_w, k_patches):
    tokens = np.asarray(tokens).astype(np.int64)
    inputs = dict(tokens=tokens, embed_W=embed_W, bp_w1=bp_w1, bp_b1=bp_b1,
                  bp_w2=bp_w2, bp_b2=bp_b2)
    zv = run_kernel_a(inputs)
    pos, pid = boundary_plan(zv, tokens)
    qtokp = np.take_along_axis(tokens, pos, 1)  # [B, K] boundary token ids

    if "B" not in _cache:
        _cache["B"] = build_kernel_b()
    nc = _cache["B"]

    bf16 = ml_dtypes.bfloat16

    def pack(a, nchunk):
        """[nchunk*128, C] -> [128, nchunk*C] chunk-column layout."""
        n, c = a.shape
        assert n == nchunk * 128
        return np.ascontiguousarray(
            a.reshape(nchunk, 128, c).transpose(1, 0, 2).reshape(128, nchunk * c))

    embp = np.zeros((VP, D), np.float32)
    embp[:V] = embed_W.astype(np.float32)
    emb_s = pack(embp, 3).astype(bf16)
    embT_s = pack(np.ascontiguousarray(embp.T), 6).astype(bf16)
    wqT_full = np.ascontiguousarray(
        (wq.astype(np.float32) * qnorm_w.astype(np.float32)[None, :]).T / 8.0)
    wkT_full = np.ascontiguousarray(
        (wk.astype(np.float32) * kvnorm_w.astype(np.float32)[None, :]).T)
    wvT_full = np.ascontiguousarray(
        (wv.astype(np.float32) * kvnorm_w.astype(np.float32)[None, :]).T)
    woT_full = np.ascontiguousarray(wo.astype(np.float32).T)

    in_maps = []
    for b in range(B):
        C = np.zeros((VP, K), np.float32)
        np.add.at(C, (tokens[b], pid[b]), 1.0)
        QOH = np.zeros((VP, K), np.float32)
        QOH[qtokp[b], np.arange(K)] = 1.0
        C_s = pack(C, 3).astype(bf16)
        QOH_s = pack(QOH, 3).astype(bf16)
        for g in range(2):
            cols = slice(DG * g, DG * (g + 1))
            in_maps.append({
                "embT": embT_s, "emb": emb_s,
                "wqT": pack(wqT_full[:, cols], 6).astype(bf16),
                "wkT": pack(wkT_full[:, cols], 6).astype(bf16),
                "wvT": pack(wvT_full[:, cols], 6).astype(bf16),
                "woT": pack(woT_full[cols, :], 3).astype(bf16),
                "cnt": C_s, "qoh": QOH_s,
            })
    res = run_bass_kernel_spmd(nc, in_maps, list(range(NCORES)),
                               trace=os.environ.get("KERNEL_TRACE") == "1")
    _cache["tB"] = res.exec_time_ns
    out = np.zeros((B, K, D), np.float32)
    for b in range(B):
        outT = res.results[2 * b]["outT"] + res.results[2 * b + 1]["outT"]
        out[b] = outT.T
    return out


# revision 14
# speedup vs baseline: 1.9849x; 1.1028x over previous
"""Trainium2 kernel for nn_LocalEncoder (BLT-style local encoder).

Key structural insight: every per-token quantity depends only on the token ID
(vocab=260), so the whole cross-attention collapses into vocab space:

  out_h(patch j) = sum_w C[w,j] * exp(S_h[w, qtok_j]) * vhat_h(w) / den
  den            = sum_w C[w,j] * exp(S_h[w, qtok_j])

with C[w,j] = count of tokens with id w inside patch j (host histogram),
S_h = khat_h^T qhat_h a (vocab x patch) score matrix, and qhat/khat/vhat the
vocab-space projection tables.  Device work per core is a handful of dense
vocab-sized matmuls -- no per-token gathers at all.

Pipeline:
  Kernel A (8 cores, DF split 8x384): zv partials = w2_slice @ silu(w1_slice @ embT)
  Host:     zv -> per-row boundary selection (stable by (-z, idx)) -> pos/pid,
            count matrix C[vocab, patch], qtok one-hot, folded weights
  Kernel B (8 cores = 4 seqs x 2 head-groups of 6): tables -> scores ->
            exp*count -> weighted-sum matmuls -> wo, partial outputs summed
            on host over the 2 head-groups.
"""

import os
import numpy as np
import ml_dtypes

import concourse.bass as bass
import concourse.bacc as bacc
import concourse.mybir as mybir
from concourse.tile import TileContext
from concourse.alu_op_type import AluOpType
from concourse.bass_utils import run_bass_kernel_spmd

F32 = mybir.dt.float32
F32R = mybir.dt.float32r
BF16 = mybir.dt.bfloat16
AFT = mybir.ActivationFunctionType
AX = mybir.AxisListType

B, L, D, V, K, H, HD = 4, 4096, 768, 260, 512, 12, 64
DF = 4 * D
VP = 384          # vocab padded to 3 partition chunks
RMS_EPS = 1e-5
NCORES = 8
FSL = DF // NCORES  # 384 f-rows per core in kernel A
DG = 384            # head-group width (6 heads x 64)

_cache = {}


# --------------------------------------------------------------------------- #
# Kernel A: per-core partial zv over a DF slice (fp32 matmuls for precision)
# --------------------------------------------------------------------------- #
def build_kernel_a():
    nc = bacc.Bacc("TRN2", target_bir_lowering=False, debug=False)
    embT_d = nc.dram_tensor("embT", [128, 6 * V], F32R, kind="ExternalInput")
    w1T_d = nc.dram_tensor("w1T", [128, 6 * FSL], F32R, kind="ExternalInput")
    bw_d = nc.dram_tensor("bw", [128, 6], F32, kind="ExternalInput")
    zp_d = nc.dram_tensor("zp", [1, V], F32, kind="ExternalOutput")

    with TileContext(nc) as tc:
        with (
            tc.tile_pool(name="sb", bufs=1) as sb,
            tc.tile_pool(name="ps", bufs=2, space="PSUM") as ps,
        ):
            embT_t = sb.tile([128, 6 * V], F32R, tag="embT", name="embT_t")
            w1T_t = sb.tile([128, 6 * FSL], F32R, tag="w1T", name="w1T_t")
            bw = sb.tile([128, 6], F32, tag="bw", name="bw")
            nc.sync.dma_start(bw[:, :], bw_d[:, :])
            for d in range(6):
                nc.sync.dma_start(embT_t[:, V * d:V * (d + 1)],
                                  embT_d[:, V * d:V * (d + 1)])
                nc.sync.dma_start(w1T_t[:, FSL * d:FSL * (d + 1)],
                                  w1T_d[:, FSL * d:FSL * (d + 1)])
            embT = [embT_t[:, V * d:V * (d + 1)] for d in range(6)]
            w1T = [w1T_t[:, FSL * d:FSL * (d + 1)] for d in range(6)]
            b1c = bw[:, 0:3]
            w2c = bw[:, 3:6]

            zp_ps = ps.tile([1, V], F32, tag="zp")
            for fi in range(3):
                y1p = ps.tile([128, V], F32, tag="y1")
                for d in range(6):
                    nc.tensor.matmul(
                        y1p[:, :], w1T_t[:, FSL * d + 128 * fi:FSL * d + 128 * (fi + 1)],
                        embT[d], start=(d == 0), stop=(d == 5),
                    )
                y1b = sb.tile([128, V], F32, tag="y1b")
                nc.vector.tensor_scalar(y1b[:, :], y1p[:, :], b1c[:, fi:fi + 1],
                                        None, AluOpType.add)
                sig = sb.tile([128, V], F32, tag="sig")
                nc.scalar.activation(sig[:, :], y1b[:, :], AFT.Sigmoid)
                y1s = sb.tile([128, V], F32, tag="y1s")
                nc.vector.tensor_tensor(y1s[:, :], y1b[:, :], sig[:, :],
                                        AluOpType.mult)
                nc.tensor.matmul(zp_ps[:, :], w2c[:, fi:fi + 1], y1s[:, :],
                                 start=(fi == 0), stop=(fi == 2))
            zp_s = sb.tile([1, V], F32, tag="zps")
            nc.vector.tensor_copy(zp_s[:, :], zp_ps[:, :])
            nc.sync.dma_start(zp_d[:, :], zp_s[:, :])
    nc.compile()
    return nc


def run_kernel_a(inputs):
    if "A" not in _cache:
        _cache["A"] = build_kernel_a()
    nc = _cache["A"]
    embT = inputs["embed_W"].astype(np.float32).T          # [768, 260]
    embT_r = np.ascontiguousarray(
        embT.reshape(6, 128, V).transpose(1, 0, 2).reshape(128, 6 * V))
    w1 = inputs["bp_w1"].astype(np.float32)
    b1 = inputs["bp_b1"].astype(np.float32)
    w2 = inputs["bp_w2"].astype(np.float32)[0]
    in_maps = []
    for c in range(NCORES):
        sl = slice(c * FSL, (c + 1) * FSL)
        w1T_r = np.ascontiguousarray(
            w1[sl].T.reshape(6, 128, FSL).transpose(1, 0, 2).reshape(128, 6 * FSL))
        bw = np.zeros((128, 6), np.float32)
        bw[:, 0:3] = b1[sl].reshape(3, 128).T
        bw[:, 3:6] = w2[sl].reshape(3, 128).T
        in_maps.append({"embT": embT_r, "w1T": w1T_r, "bw": bw})
    res = run_bass_kernel_spmd(nc, in_maps, list(range(NCORES)),
                               trace=os.environ.get("KERNEL_TRACE") == "1")
    _cache["tA"] = res.exec_time_ns
    zv = np.zeros(V, np.float64)
    for c in range(NCORES):
        zv += res.results[c]["zp"][0].astype(np.float64)
    zv += inputs["bp_b2"].astype(np.float64)[0]
    return zv.astype(np.float32)


# --------------------------------------------------------------------------- #
# Host boundary logic
# --------------------------------------------------------------------------- #
def boundary_plan(zv, tokens):
    """Reproduce reference top-k (stable ties by index) + patch structure."""
    zt = zv[tokens]  # [B, L]
    pos = np.zeros((B, K), np.int64)
    for b in range(B):
        key = zt[b].astype(np.float64).copy()
        key[0] = np.inf  # position 0 forced boundary (logprob set to 0 = max)
        order = np.lexsort((np.arange(L), -key))
        pos[b] = np.sort(order[:K])
    pid = (pos[:, None, :] <= np.arange(L)[None, :, None]).sum(-1) - 1  # [B, L]
    return pos, pid


# --------------------------------------------------------------------------- #
# Kernel B: count-matrix vocab-space cross attention, 6 heads per core
# --------------------------------------------------------------------------- #
def strided3(ap, n, w, stride, offset=0):
    """[128, *] AP viewed as [128, n, w] blocks at `offset` with block stride."""
    ps = ap.ap[0]
    return bass.AP(ap.tensor, ap.offset + offset,
                   [list(ps), [stride, n], [1, w]])


def build_kernel_b():
    nc = bacc.Bacc("TRN2", target_bir_lowering=False, debug=False)
    embT_d = nc.dram_tensor("embT", [128, 6 * VP], BF16, kind="ExternalInput")
    emb_d = nc.dram_tensor("emb", [128, 3 * D], BF16, kind="ExternalInput")
    wqT_d = nc.dram_tensor("wqT", [128, 6 * DG], BF16, kind="ExternalInput")
    wkT_d = nc.dram_tensor("wkT", [128, 6 * DG], BF16, kind="ExternalInput")
    wvT_d = nc.dram_tensor("wvT", [128, 6 * DG], BF16, kind="ExternalInput")
    woT_d = nc.dram_tensor("woT", [128, 3 * D], BF16, kind="ExternalInput")
    c_d = nc.dram_tensor("cnt", [128, 3 * K], BF16, kind="ExternalInput")
    qoh_d = nc.dram_tensor("qoh", [128, 3 * K], BF16, kind="ExternalInput")
    outT_d = nc.dram_tensor("outT", [D, K], F32, kind="ExternalOutput")

    with TileContext(nc) as tc:
        with (
            tc.tile_pool(name="sb", bufs=1) as sb,
            tc.tile_pool(name="ps", bufs=2, space="PSUM") as ps,
        ):
            # ---- loads (one DMA per logical tensor) ----
            embT_t = sb.tile([128, 6 * VP], BF16, tag="embT", name="embT_t")
            wqT_t = sb.tile([128, 6 * DG], BF16, tag="wqT", name="wqT_t")
            wkT_t = sb.tile([128, 6 * DG], BF16, tag="wkT", name="wkT_t")
            wvT_t = sb.tile([128, 6 * DG], BF16, tag="wvT", name="wvT_t")
            emb_t = sb.tile([128, 3 * D], BF16, tag="emb", name="emb_t")
            c_t = sb.tile([128, 3 * K], BF16, tag="ct", name="c_t")
            qoh_t = sb.tile([128, 3 * K], BF16, tag="qoht", name="qoh_t")
            wo_t = sb.tile([128, 3 * D], BF16, tag="wot", name="wo_t")
            for d in range(6):
                nc.sync.dma_start(embT_t[:, VP * d:VP * (d + 1)],
                                  embT_d[:, VP * d:VP * (d + 1)])
                nc.sync.dma_start(wqT_t[:, DG * d:DG * (d + 1)],
                                  wqT_d[:, DG * d:DG * (d + 1)])
            for u in range(3):
                nc.sync.dma_start(emb_t[:, D * u:D * (u + 1)],
                                  emb_d[:, D * u:D * (u + 1)])
            for d in range(6):
                nc.sync.dma_start(wkT_t[:, DG * d:DG * (d + 1)],
                                  wkT_d[:, DG * d:DG * (d + 1)])
                nc.sync.dma_start(wvT_t[:, DG * d:DG * (d + 1)],
                                  wvT_d[:, DG * d:DG * (d + 1)])
            for u in range(3):
                nc.sync.dma_start(qoh_t[:, K * u:K * (u + 1)],
                                  qoh_d[:, K * u:K * (u + 1)])
                nc.sync.dma_start(c_t[:, K * u:K * (u + 1)],
                                  c_d[:, K * u:K * (u + 1)])
            for u in range(3):
                nc.sync.dma_start(wo_t[:, D * u:D * (u + 1)],
                                  woT_d[:, D * u:D * (u + 1)])
            embT6 = [embT_t[:, VP * d:VP * (d + 1)] for d in range(6)]
            wqT6 = [wqT_t[:, DG * d:DG * (d + 1)] for d in range(6)]
            wkT6 = [wkT_t[:, DG * d:DG * (d + 1)] for d in range(6)]
            wvT6 = [wvT_t[:, DG * d:DG * (d + 1)] for d in range(6)]
            emb3 = [emb_t[:, D * u:D * (u + 1)] for u in range(3)]
            C3 = [c_t[:, K * u:K * (u + 1)] for u in range(3)]
            QOH3 = [qoh_t[:, K * u:K * (u + 1)] for u in range(3)]
            woT3 = [wo_t[:, D * u:D * (u + 1)] for u in range(3)]
            ones64 = sb.tile([128, 64], BF16, tag="ones64")
            nc.gpsimd.memset(ones64[:, :], 1.0)

            # ---- rmsnorm scales rv[u] = rsqrt(mean(emb_u^2) + eps) ----
            rv3 = []
            for u in range(3):
                sq = sb.tile([128, D], BF16, tag="sq", name="sq", bufs=2)
                nc.gpsimd.tensor_tensor(sq[:, :], emb3[u][:, :], emb3[u][:, :],
                                        AluOpType.mult)
                msq = sb.tile([128, 1], F32, tag="msq", name="msq", bufs=2)
                nc.vector.tensor_reduce(msq[:, :], sq[:, :], AX.X, AluOpType.add)
                tn = sb.tile([128, 1], F32, tag="tn", name="tn", bufs=2)
                nc.vector.tensor_scalar(tn[:, :], msq[:, :], 1.0 / D, RMS_EPS,
                                        AluOpType.mult, AluOpType.add)
                tr = sb.tile([128, 1], F32, tag="tr", name="tr", bufs=2)
                nc.vector.reciprocal(tr[:, :], tn[:, :])
                rv = sb.tile([128, 1], F32, tag=f"rv{u}", name=f"rv{u}")
                nc.scalar.activation(rv[:, :], tr[:, :], AFT.Sqrt)
                rv3.append(rv)

            # ---- tables ----
            qhat3 = []   # [u-chunk][128, DG] rv-scaled (1/8 folded on host)
            for u in range(3):
                tp = ps.tile([128, K], F32, tag="w512", name="tpq")
                for d in range(6):
                    nc.tensor.matmul(tp[:, :DG], embT6[d][:, 128 * u:128 * (u + 1)],
                                     wqT6[d][:, :], start=(d == 0), stop=(d == 5))
                qs = sb.tile([128, DG], BF16, tag=f"qhat{u}", name=f"qhat{u}")
                nc.vector.tensor_scalar(qs[:, :], tp[:, :DG], rv3[u][:, 0:1],
                                        None, AluOpType.mult)
                qhat3.append(qs)
            ktT3 = []    # [r-chunk][128, VP] un-scaled (rv applied inside exp)
            for r in range(3):
                tp = ps.tile([128, K], F32, tag="w512", name="tpk")
                for d in range(6):
                    nc.tensor.matmul(tp[:, :VP], wkT6[d][:, 128 * r:128 * (r + 1)],
                                     embT6[d][:, :], start=(d == 0), stop=(d == 5))
                ks = sb.tile([128, VP], BF16, tag=f"ktT{r}", name=f"ktT{r}")
                nc.scalar.copy(ks[:, :], tp[:, :VP])
                ktT3.append(ks)
            vhat3 = []   # [u-chunk][128, 390]: per head 64 cols + ones col
            for u in range(3):
                tp = ps.tile([128, K], F32, tag="w512", name="tpv")
                for d in range(6):
                    nc.tensor.matmul(tp[:, :DG], embT6[d][:, 128 * u:128 * (u + 1)],
                                     wvT6[d][:, :], start=(d == 0), stop=(d == 5))
                vt = sb.tile([128, 390], BF16, tag=f"vhat{u}", name=f"vhat{u}")
                nc.gpsimd.memset(strided3(vt[:, :], 6, 1, 65, offset=64), 1.0)
                nc.vector.tensor_scalar(strided3(vt[:, :], 6, 64, 65),
                                        strided3(tp[:, :DG], 6, 64, 64),
                                        rv3[u][:, 0:1], None, AluOpType.mult)
                vhat3.append(vt)

            # ---- qgT[r, j]: q rows gathered per patch by qtok one-hot ----
            qgT3 = []
            for r in range(3):
                qp = ps.tile([128, K], F32, tag="w512", name="qp")
                for u in range(3):
                    nc.tensor.matmul(qp[:, :], qhat3[u][:, 128 * r:128 * (r + 1)],
                                     QOH3[u][:, :], start=(u == 0), stop=(u == 2))
                qg = sb.tile([128, K], BF16, tag=f"qgT{r}", name=f"qgT{r}")
                nc.vector.tensor_copy(qg[:, :], qp[:, :])
                qgT3.append(qg)

            # ---- attention: per head scores -> exp*count -> num/den ----
            prT3 = [sb.tile([128, K], BF16, tag=f"prT{r}", name=f"prT{r}") for r in range(3)]
            for h in range(6):
                r, off = h // 2, 64 * (h % 2)
                nm = ps.tile([128, K], F32, tag="num", name="nm")
                for w in range(3):
                    sp = ps.tile([128, K], F32, tag="w512", name="sp")
                    nc.tensor.matmul(sp[:, :],
                                     ktT3[r][off:off + 64, 128 * w:128 * (w + 1)],
                                     qgT3[r][off:off + 64, :],
                                     start=True, stop=True)
                    ex = sb.tile([128, K], BF16, tag="ex", name="ex", bufs=3)
                    nc.scalar.activation(ex[:, :], sp[:, :], AFT.Exp,
                                         scale=rv3[w][:, 0:1])
                    xt = sb.tile([128, K], BF16, tag="xt", name="xt", bufs=3)
                    nc.vector.tensor_tensor(xt[:, :], ex[:, :], C3[w][:, :],
                                            AluOpType.mult)
                    nc.tensor.matmul(nm[0:65, :], vhat3[w][:, 65 * h:65 * h + 65],
                                     xt[:, :], start=(w == 0), stop=(w == 2))
                rdf = sb.tile([128, K], F32, tag="rdf", name="rdf", bufs=2)
                nc.vector.reciprocal_approx_fast(rdf[:, :], nm[:, :])
                rd = sb.tile([128, K], BF16, tag="rd", name="rd", bufs=2)
                nc.vector.tensor_copy(rd[64:65, :], rdf[64:65, :])
                pp = ps.tile([128, K], F32, tag="pp", name="pp")
                nc.tensor.matmul(pp[0:64, :], ones64[64:65, :], rd[64:65, :],
                                 start=True, stop=True)
                nmsb = sb.tile([64, K], BF16, tag="nmsb", name="nmsb", bufs=2)
                nc.scalar.copy(nmsb[:, :], nm[0:64, :])
                if off == 0:
                    nc.vector.tensor_tensor(prT3[r][0:64, :], nmsb[:, :],
                                            pp[0:64, :], AluOpType.mult)
                else:
                    po = sb.tile([64, K], BF16, tag="po", name="po", bufs=2)
                    nc.vector.tensor_tensor(po[:, :], nmsb[:, :], pp[0:64, :],
                                            AluOpType.mult)
                    nc.sync.dma_start(prT3[r][64:128, :], po[:, :])

            # ---- wo projection (transposed output) ----
            for m in range(6):
                op = ps.tile([128, K], F32, tag="w512", name="op")
                for kc in range(3):
                    nc.tensor.matmul(op[:, :], woT3[kc][:, 128 * m:128 * (m + 1)],
                                     prT3[kc][:, :], start=(kc == 0), stop=(kc == 2))
                ot = sb.tile([128, K], F32, tag="ot", name="ot", bufs=2)
                nc.vector.tensor_copy(ot[:, :], op[:, :])
                nc.sync.dma_start(outT_d[128 * m:128 * (m + 1), :], ot[:, :])
    nc.compile()
    return nc


# --------------------------------------------------------------------------- #
# top-level
# --------------------------------------------------------------------------- #
def kernel(tokens, embed_W, bp_w1, bp_b1, bp_w2, bp_b2, wq, wk, wv, wo,
           qnorm_w, kvnorm_w, k_patches):
    tokens = np.asarray(tokens).astype(np.int64)
    inputs = dict(tokens=tokens, embed_W=embed_W, bp_w1=bp_w1, bp_b1=bp_b1,
                  bp_w2=bp_w2, bp_b2=bp_b2)
    zv = run_kernel_a(inputs)
    pos, pid = boundary_plan(zv, tokens)
    qtokp = np.take_along_axis(tokens, pos, 1)  # [B, K] boundary token ids

    if "B" not in _cache:
        _cache["B"] = build_kernel_b()
    nc = _cache["B"]

    bf16 = ml_dtypes.bfloat16

    def pack(a, nchunk):
        """[nchunk*128, C] -> [128, nchunk*C] chunk-column layout."""
        n, c = a.shape
        assert n == nchunk * 128
        return np.ascontiguousarray(
            a.reshape(nchunk, 128, c).transpose(1, 0, 2).reshape(128, nchunk * c))

    embp = np.zeros((VP, D), np.float32)
    embp[:V] = embed_W.astype(np.float32)
    emb_s = pack(embp, 3).astype(bf16)
    embT_s = pack(np.ascontiguousarray(embp.T), 6).astype(bf16)
    wqT_full = np.ascontiguousarray(
        (wq.astype(np.float32) * qnorm_w.astype(np.float32)[None, :]).T / 8.0)
    wkT_full = np.ascontiguousarray(
        (wk.astype(np.float32) * kvnorm_w.astype(np.float32)[None, :]).T)
    wvT_full = np.ascontiguousarray(
        (wv.astype(np.float32) * kvnorm_w.astype(np.float32)[None, :]).T)
    woT_full = np.ascontiguousarray(wo.astype(np.float32).T)

    in_maps = []
    for b in range(B):
        C = np.zeros((VP, K), np.float32)
        np.add.at(C, (tokens[b], pid[b]), 1.0)
        QOH = np.zeros((VP, K), np.float32)
        QOH[qtokp[b], np.arange(K)] = 1.0
        C_s = pack(C, 3).astype(bf16)
        QOH_s = pack(QOH, 3).astype(bf16)
        for g in range(2):
            cols = slice(DG * g, DG * (g + 1))
            in_maps.append({
                "embT": embT_s, "emb": emb_s,
                "wqT": pack(wqT_full[:, cols], 6).astype(bf16),
                "wkT": pack(wkT_full[:, cols], 6).astype(bf16),
                "wvT": pack(wvT_full[:, cols], 6).astype(bf16),
                "woT": pack(woT_full[cols, :], 3).astype(bf16),
                "cnt": C_s, "qoh": QOH_s,
            })
    res = run_bass_kernel_spmd(nc, in_maps, list(range(NCORES)),
                               trace=os.environ.get("KERNEL_TRACE") == "1")
    _cache["tB"] = res.exec_time_ns
    out = np.zeros((B, K, D), np.float32)
    for b in range(B):
        outT = res.results[2 * b]["outT"] + res.results[2 * b + 1]["outT"]
        out[b] = outT.T
    return out
